# revision 6
# baseline (speedup 1.0000x reference)
"""Distributed Trainium2 kernel for the dense-transformer attention block:

    xn = LN(x); q,k = xn @ w_qk; v = xn @ w_v
    out = softmax(q k^T / sqrt(hd)) v ; out = LN(out) @ w_proj + b_proj

Sharding: the (B=2, N=2048) token axis is flattened to 4096 rows and split
512 rows per core (cores 0-3 own batch 0, cores 4-7 batch 1).  Each core
runs LN1 + the K/V projections on its rows, all-gathers K^T/V inside its
4-core batch group, then computes all 16 heads of attention for its own
512 query rows, LN2 and the output projection — so every FLOP except the
gather is done exactly once across the chip.

Everything on-chip lives in a transposed ("feature on partition") layout:
the host feeds x^T per core and transposes the returned y^T back, which
costs nothing on hardware.  All matmuls run in fp32r (fp32 with an
11-bit mantissa, full PE rate); weights are pre-rounded on the host.
"""

import numpy as np

import concourse.bass as bass
import concourse.mybir as mybir
import concourse.tile as tile
from concourse import bacc
from concourse.bass_utils import run_bass_kernel_spmd

B, N, C = 2, 2048, 1024
H, HD = 16, 64
NCORES = 8
R = (B * N) // NCORES  # 512 rows per core
GROUP = NCORES // B  # 4 cores per batch
EPS = 1e-5
SCALE = HD**-0.5

F32 = mybir.dt.float32
F32R = mybir.dt.float32r
AF = mybir.ActivationFunctionType
ALU = mybir.AluOpType

CT = C // 128  # 8 C tiles
KV_K_ELEMS = C * R  # k^T region elems per core
KV_ELEMS = 2 * KV_K_ELEMS  # k^T + v

last_exec_time_ns = None
_cached_nc = None


def _round_fp32r(a: np.ndarray) -> np.ndarray:
    bits = np.ascontiguousarray(a, dtype=np.float32).view(np.uint32)
    lsb = (bits >> 12) & 1
    rounded = (bits + 0x7FF + lsb) & np.uint32(0xFFFFF000)
    return rounded.view(np.float32)


def _build():
    nc = bacc.Bacc("TRN2", target_bir_lowering=False, debug=False, num_devices=NCORES)

    xT_ext = nc.dram_tensor("xT", [C, R], F32R, kind="ExternalInput")
    wqk_ext = nc.dram_tensor("w_qk", [C, 2 * C], F32R, kind="ExternalInput")
    wv_ext = nc.dram_tensor("w_v", [C, C], F32R, kind="ExternalInput")
    wp_ext = nc.dram_tensor("w_proj", [C, C], F32R, kind="ExternalInput")
    g1_ext = nc.dram_tensor("ln1_g", [C], F32, kind="ExternalInput")
    b1_ext = nc.dram_tensor("ln1_b", [C], F32, kind="ExternalInput")
    g2_ext = nc.dram_tensor("ln2_g", [C], F32, kind="ExternalInput")
    b2_ext = nc.dram_tensor("ln2_b", [C], F32, kind="ExternalInput")
    bp_ext = nc.dram_tensor("b_proj", [C], F32, kind="ExternalInput")
    out_ext = nc.dram_tensor("out", [C, R], F32, kind="ExternalOutput")

    kv_in = nc.dram_tensor("kv_in", [KV_ELEMS], F32)
    kv_out = nc.dram_tensor("kv_out", [GROUP, KV_ELEMS], F32)

    def kT_region(ap):  # [C, R] feature-major k^T
        return ap[0:KV_K_ELEMS].rearrange("(f r) -> f r", r=R)

    def v_region(ap):  # [R, C] row-major v
        return ap[KV_K_ELEMS:KV_ELEMS].rearrange("(r f) -> r f", f=C)

    ones_col_np = np.ones((128, 1), np.float32)
    ones_row_np = np.ones((1, 128), np.float32)
    ones_h_np = np.ones((128, H), np.float32)

    with tile.TileContext(nc) as tc:
        with (
            tc.tile_pool(name="const", bufs=1) as constp,
            tc.tile_pool(name="qT", bufs=1) as qTp,
            tc.tile_pool(name="o", bufs=1) as op_,
        ):
            ones_col = constp.tile([128, 1], F32R, tag="onesc")
            nc.sync.dma_start(
                ones_col[:], nc.inline_tensor(ones_col_np, "onesc").ap().bitcast(F32R)
            )
            ones_row = constp.tile([1, 128], F32R, tag="onesr")
            nc.sync.dma_start(
                ones_row[:], nc.inline_tensor(ones_row_np, "onesr").ap().bitcast(F32R)
            )
            g1 = constp.tile([128, CT], F32, tag="g1")
            nc.sync.dma_start(g1[:], g1_ext.ap().rearrange("(c p) -> p c", p=128))
            b1 = constp.tile([128, CT], F32, tag="b1")
            nc.sync.dma_start(b1[:], b1_ext.ap().rearrange("(c p) -> p c", p=128))
            g2h = constp.tile([64, H], F32, tag="g2h")
            nc.sync.dma_start(g2h[:], g2_ext.ap().rearrange("(h p) -> p h", p=64))
            b2h = constp.tile([64, H], F32, tag="b2h")
            nc.sync.dma_start(b2h[:], b2_ext.ap().rearrange("(h p) -> p h", p=64))
            bp = constp.tile([128, CT], F32, tag="bp")
            nc.sync.dma_start(bp[:], bp_ext.ap().rearrange("(c p) -> p c", p=128))
            ones_h_dram = nc.inline_tensor(ones_h_np, "onesh")
            eps_t = constp.tile([1, 1], F32, tag="epsc")
            nc.sync.dma_start(
                eps_t[:], nc.inline_tensor(np.full((1, 1), EPS, np.float32), "epsc").ap()
            )

            qT = [qTp.tile([128, R], F32R, tag=f"qT{p}", name=f"qT{p}") for p in range(H // 2)]
            o = [op_.tile([64, R], F32R, tag=f"o{h}", name=f"o{h}") for h in range(H)]

            # ---------------- Phase 1: LN1 + K/V/Q projections ----------
            with (
                tc.tile_pool(name="xn", bufs=1) as xnp,
                tc.tile_pool(name="tmp1", bufs=3) as tmp1p,
                tc.tile_pool(name="small1", bufs=1) as small1p,
                tc.tile_pool(name="ev1", bufs=4) as ev1p,
                tc.tile_pool(name="ps_stat", bufs=1, space="PSUM") as ps_stat,
                tc.tile_pool(name="ps_bc", bufs=1, space="PSUM") as ps_bc,
                tc.tile_pool(name="ps_mm", bufs=3, space="PSUM") as ps_mm,
                tc.tile_pool(name="xt", bufs=1) as xtp,
                tc.tile_pool(name="w1", bufs=1) as w1p,
                tc.tile_pool(name="wk", bufs=16) as wkp,
            ):
                xt = []
                for c in range(CT):
                    t = xtp.tile([128, R], F32R, tag=f"xt{c}")
                    nc.sync.dma_start(t[:], xT_ext[128 * c : 128 * (c + 1), :])
                    xt.append(t)

                # LN1 stats: sums of x and x^2 over C (partition axis)
                sx_ps = ps_stat.tile([1, R], F32, tag="sx")
                sq_ps = ps_stat.tile([1, R], F32, tag="sq")
                for c in range(CT):
                    nc.tensor.matmul(
                        sx_ps[:], ones_col[:], xt[c][:], start=(c == 0), stop=(c == CT - 1)
                    )
                for c in range(CT):
                    xsq = tmp1p.tile([128, R], F32R, tag="xsq")
                    nc.scalar.activation(xsq[:], xt[c][:], AF.Square)
                    nc.tensor.matmul(
                        sq_ps[:], ones_col[:], xsq[:], start=(c == 0), stop=(c == CT - 1)
                    )

                mu = small1p.tile([1, R], F32R, tag="mu")
                nc.vector.tensor_scalar_mul(mu[:], sx_ps[:], 1.0 / C)
                m2 = small1p.tile([1, R], F32, tag="m2")
                nc.vector.tensor_scalar_mul(m2[:], sq_ps[:], 1.0 / C)
                musq = small1p.tile([1, R], F32, tag="musq")
                nc.vector.tensor_tensor(musq[:], mu[:], mu[:], op=ALU.mult)
                var = small1p.tile([1, R], F32, tag="var")
                nc.vector.tensor_tensor(var[:], m2[:], musq[:], op=ALU.subtract)
                lv = small1p.tile([1, R], F32, tag="lv")
                nc.scalar.activation(lv[:], var[:], AF.Ln, bias=eps_t[:])
                rsig = small1p.tile([1, R], F32R, tag="rsig")
                nc.scalar.activation(rsig[:], lv[:], AF.Exp, scale=-0.5)

                bmu_ps = ps_bc.tile([128, R], F32, tag="bmu")
                nc.tensor.matmul(bmu_ps[:], ones_row[:], mu[:], start=True, stop=True)
                brs_ps = ps_bc.tile([128, R], F32, tag="brs")
                nc.tensor.matmul(brs_ps[:], ones_row[:], rsig[:], start=True, stop=True)

                xn = []
                for c in range(CT):
                    t1 = tmp1p.tile([128, R], F32, tag="lt1")
                    nc.vector.tensor_tensor(t1[:], xt[c][:], bmu_ps[:], op=ALU.subtract)
                    t2 = tmp1p.tile([128, R], F32, tag="lt2")
                    nc.vector.tensor_tensor(t2[:], t1[:], brs_ps[:], op=ALU.mult)
                    t3 = xnp.tile([128, R], F32R, tag=f"xn{c}")
                    nc.vector.tensor_scalar(
                        t3[:], t2[:], g1[:, c : c + 1], b1[:, c : c + 1],
                        op0=ALU.mult, op1=ALU.add,
                    )
                    xn.append(t3)

                # resident w_v slabs (reused across row tiles)
                wv_sb = []
                for c in range(CT):
                    t = w1p.tile([128, C], F32R, tag=f"wv{c}", name=f"wv{c}")
                    nc.sync.dma_start(t[:], wv_ext[128 * c : 128 * (c + 1), :])
                    wv_sb.append(t)

                # k^T = w_k^T @ xn^T   -> [C, R] feature-major, into kv bounce
                # (w_qk tiles are streamed: each is used exactly once)
                for kf in range(CT):
                    kps = ps_mm.tile([128, R], F32, tag="kvps")
                    for c in range(CT):
                        wt = wkp.tile([128, 128], F32R, tag="wk")
                        nc.sync.dma_start(
                            wt[:],
                            wqk_ext[
                                128 * c : 128 * (c + 1),
                                C + 128 * kf : C + 128 * (kf + 1),
                            ],
                        )
                        nc.tensor.matmul(
                            kps[:], wt[:], xn[c][:],
                            start=(c == 0), stop=(c == CT - 1),
                        )
                    ksb = ev1p.tile([128, R], F32R, tag="kev")
                    nc.scalar.copy(ksb[:], kps[:])
                    nc.sync.dma_start(
                        kT_region(kv_in.ap())[128 * kf : 128 * (kf + 1), :].bitcast(F32R),
                        ksb[:],
                    )

                # v = xn @ w_v -> [R, C] row-major, into kv bounce
                for rt in range(R // 128):
                    for vf in range(2):
                        vps = ps_mm.tile([128, R], F32, tag="kvps")
                        for c in range(CT):
                            nc.tensor.matmul(
                                vps[:],
                                xn[c][:, 128 * rt : 128 * (rt + 1)],
                                wv_sb[c][:, 512 * vf : 512 * (vf + 1)],
                                start=(c == 0),
                                stop=(c == CT - 1),
                            )
                        vsb = ev1p.tile([128, R], F32R, tag="vev")
                        nc.scalar.copy(vsb[:], vps[:])
                        nc.sync.dma_start(
                            v_region(kv_in.ap())[
                                128 * rt : 128 * (rt + 1), 512 * vf : 512 * (vf + 1)
                            ].bitcast(F32R),
                            vsb[:],
                        )

                # ---- the one collective: gather k^T/v across the batch group
                nc.gpsimd.collective_compute(
                    "AllGather",
                    ALU.bypass,
                    ins=[kv_in[:]],
                    outs=[kv_out[:]],
                    replica_groups=[[0, 1, 2, 3], [4, 5, 6, 7]],
                )

                # q^T = w_q^T @ xn^T -> [C, R] (overlaps the all-gather)
                for qf in range(CT):
                    qps = ps_mm.tile([128, R], F32, tag="kvps")
                    for c in range(CT):
                        wt = wkp.tile([128, 128], F32R, tag="wk")
                        nc.sync.dma_start(
                            wt[:],
                            wqk_ext[
                                128 * c : 128 * (c + 1), 128 * qf : 128 * (qf + 1)
                            ],
                        )
                        nc.tensor.matmul(
                            qps[:], wt[:], xn[c][:],
                            start=(c == 0), stop=(c == CT - 1),
                        )
                    nc.scalar.copy(qT[qf][:], qps[:])

            # ---------------- Phase 2: attention, one head-pair at a time
            NKC = (GROUP * R) // 128  # 16 k chunks of 128
            with (
                tc.tile_pool(name="kpair", bufs=2) as kpp,
                tc.tile_pool(name="vaug", bufs=3) as vap,
                tc.tile_pool(name="att", bufs=6) as attp,
                tc.tile_pool(name="rec", bufs=4) as recp,
                tc.tile_pool(name="ps_sc", bufs=2, space="PSUM") as ps_sc,
                tc.tile_pool(name="ps_av", bufs=2, space="PSUM") as ps_av,
                tc.tile_pool(name="ps_nb", bufs=2, space="PSUM") as ps_nb,
            ):
                for p in range(H // 2):
                    kpair = kpp.tile([128, GROUP * R], F32R, tag="kpair")
                    for r in range(GROUP):
                        nc.sync.dma_start(
                            kpair[:, R * r : R * (r + 1)],
                            kT_region(kv_out[r])[128 * p : 128 * (p + 1), :].bitcast(F32R),
                        )
                    vaug = []
                    for hi in range(2):
                        h = 2 * p + hi
                        va = vap.tile([128, NKC, HD + 1], F32R, tag="vaug")
                        for r in range(GROUP):
                            nc.sync.dma_start(
                                va[:, 4 * r : 4 * (r + 1), 0:HD],
                                v_region(kv_out[r])
                                .rearrange("(j p) f -> p j f", p=128)[
                                    :, :, HD * h : HD * (h + 1)
                                ]
                                .bitcast(F32R),
                            )
                        nc.sync.dma_start(
                            va[:, :, HD : HD + 1], ones_h_dram.ap().bitcast(F32R)
                        )
                        vaug.append(va)

                    # scores + exp, interleaving the two heads (PE row groups
                    # 0-63 / 64-127 run concurrently)
                    att = [[], []]
                    for g in range(NKC // 2):
                        scp = [None, None]
                        for hi in range(2):
                            base = 64 * hi
                            scp[hi] = ps_sc.tile([128, 1024], F32, tag="scps", name=f"scps{p}_{g}_{hi}")
                            for cc in range(2):
                                j = 2 * g + cc
                                nc.tensor.matmul(
                                    scp[hi][:, 512 * cc : 512 * (cc + 1)],
                                    kpair[base : base + 64, 128 * j : 128 * (j + 1)],
                                    qT[p][base : base + 64, :],
                                    start=True,
                                    stop=True,
                                )
                        for hi in range(2):
                            at = attp.tile([128, 1024], F32R, tag="att")
                            nc.scalar.activation(at[:], scp[hi][:], AF.Exp, scale=SCALE)
                            att[hi].append(at)

                    for hi in range(2):
                        h = 2 * p + hi
                        avps = ps_av.tile([HD + 1, R], F32, tag="avps")
                        for j in range(NKC):
                            nc.tensor.matmul(
                                avps[:],
                                vaug[hi][:, j, :],
                                att[hi][j // 2][:, 512 * (j % 2) : 512 * (j % 2 + 1)],
                                start=(j == 0),
                                stop=(j == NKC - 1),
                            )
                        rec = recp.tile([1, R], F32R, tag="rec")
                        with nc.allow_low_precision(reason="softmax denom bcast"):
                            nc.vector.reciprocal(rec[:], avps[HD : HD + 1, :])
                        nbps = ps_nb.tile([64, R], F32, tag="nbps")
                        nc.tensor.matmul(
                            nbps[:], ones_row[:, 0:64], rec[:], start=True, stop=True
                        )
                        nc.vector.tensor_copy(o[h][:], avps[0:HD, :])
                        nc.vector.tensor_tensor(o[h][:], o[h][:], nbps[:], op=ALU.mult)

            # ---------------- Phase 3: LN2 + projection ------------------
            with (
                tc.tile_pool(name="wp", bufs=1) as wpp,
                tc.tile_pool(name="tmp2", bufs=4) as tmp2p,
                tc.tile_pool(name="small2", bufs=1) as small2p,
                tc.tile_pool(name="ln2o", bufs=1) as ln2op,
                tc.tile_pool(name="yev", bufs=3) as yevp,
                tc.tile_pool(name="ps_stat2", bufs=1, space="PSUM") as ps_stat2,
                tc.tile_pool(name="ps_bc2", bufs=1, space="PSUM") as ps_bc2,
                tc.tile_pool(name="ps_y", bufs=3, space="PSUM") as ps_y,
            ):
                wp_sb = []
                for h in range(H):
                    t = wpp.tile([64, C], F32R, tag=f"wp{h}")
                    nc.sync.dma_start(t[:], wp_ext[HD * h : HD * (h + 1), :])
                    wp_sb.append(t)

                s2_ps = ps_stat2.tile([1, R], F32, tag="s2")
                q2_ps = ps_stat2.tile([1, R], F32, tag="q2")
                for h in range(H):
                    nc.tensor.matmul(
                        s2_ps[:], ones_col[0:64, :], o[h][:],
                        start=(h == 0), stop=(h == H - 1),
                    )
                for h in range(H):
                    osq = tmp2p.tile([64, R], F32R, tag="osq")
                    nc.scalar.activation(osq[:], o[h][:], AF.Square)
                    nc.tensor.matmul(
                        q2_ps[:], ones_col[0:64, :], osq[:],
                        start=(h == 0), stop=(h == H - 1),
                    )

                mu2 = small2p.tile([1, R], F32R, tag="mu2")
                nc.vector.tensor_scalar_mul(mu2[:], s2_ps[:], 1.0 / C)
                m22 = small2p.tile([1, R], F32, tag="m22")
                nc.vector.tensor_scalar_mul(m22[:], q2_ps[:], 1.0 / C)
                musq2 = small2p.tile([1, R], F32, tag="musq2")
                nc.vector.tensor_tensor(musq2[:], mu2[:], mu2[:], op=ALU.mult)
                var2 = small2p.tile([1, R], F32, tag="var2")
                nc.vector.tensor_tensor(var2[:], m22[:], musq2[:], op=ALU.subtract)
                lv2 = small2p.tile([1, R], F32, tag="lv2")
                nc.scalar.activation(lv2[:], var2[:], AF.Ln, bias=eps_t[:])
                rsig2 = small2p.tile([1, R], F32R, tag="rsig2")
                nc.scalar.activation(rsig2[:], lv2[:], AF.Exp, scale=-0.5)

                bmu2_ps = ps_bc2.tile([64, R], F32, tag="bmu2")
                nc.tensor.matmul(
                    bmu2_ps[:], ones_row[:, 0:64], mu2[:], start=True, stop=True
                )
                brs2_ps = ps_bc2.tile([64, R], F32, tag="brs2")
                nc.tensor.matmul(
                    brs2_ps[:], ones_row[:, 0:64], rsig2[:], start=True, stop=True
                )

                ln2o = []
                for h in range(H):
                    t1 = tmp2p.tile([64, R], F32, tag="l2t1")
                    nc.vector.tensor_tensor(t1[:], o[h][:], bmu2_ps[:], op=ALU.subtract)
                    t2 = tmp2p.tile([64, R], F32, tag="l2t2")
                    nc.vector.tensor_tensor(t2[:], t1[:], brs2_ps[:], op=ALU.mult)
                    t3 = ln2op.tile([64, R], F32R, tag=f"ln2o{h}")
                    nc.vector.tensor_scalar(
                        t3[:], t2[:], g2h[:, h : h + 1], b2h[:, h : h + 1],
                        op0=ALU.mult, op1=ALU.add,
                    )
                    ln2o.append(t3)

                for of in range(CT):
                    yps = ps_y.tile([128, R], F32, tag="yps")
                    for h in range(H):
                        nc.tensor.matmul(
                            yps[:],
                            wp_sb[h][:, 128 * of : 128 * (of + 1)],
                            ln2o[h][:],
                            start=(h == 0),
                            stop=(h == H - 1),
                        )
                    ysb = yevp.tile([128, R], F32, tag="yev")
                    nc.scalar.activation(
                        ysb[:], yps[:], AF.Identity, bias=bp[:, of : of + 1]
                    )
                    nc.sync.dma_start(out_ext[128 * of : 128 * (of + 1), :], ysb[:])

    nc.compile()
    return nc


def kernel(x, ln1_g, ln1_b, w_qk, w_v, ln2_g, ln2_b, w_proj, b_proj):
    global _cached_nc, last_exec_time_ns
    if _cached_nc is None:
        _cached_nc = _build()
    nc = _cached_nc

    x = np.asarray(x, dtype=np.float32)
    xr = _round_fp32r(x.reshape(B * N, C))
    wqk_r = _round_fp32r(np.asarray(w_qk, np.float32))
    wv_r = _round_fp32r(np.asarray(w_v, np.float32))
    wp_r = _round_fp32r(np.asarray(w_proj, np.float32))

    shared = {
        "w_qk": wqk_r,
        "w_v": wv_r,
        "w_proj": wp_r,
        "ln1_g": np.ascontiguousarray(ln1_g, np.float32),
        "ln1_b": np.ascontiguousarray(ln1_b, np.float32),
        "ln2_g": np.ascontiguousarray(ln2_g, np.float32),
        "ln2_b": np.ascontiguousarray(ln2_b, np.float32),
        "b_proj": np.ascontiguousarray(b_proj, np.float32),
    }
    in_maps = []
    for i in range(NCORES):
        xT_i = np.ascontiguousarray(xr[R * i : R * (i + 1), :].T)
        in_maps.append({"xT": xT_i, **shared})

    res = run_bass_kernel_spmd(nc, in_maps, core_ids=list(range(NCORES)))
    last_exec_time_ns = res.exec_time_ns

    y = np.empty((B * N, C), np.float32)
    for i in range(NCORES):
        y[R * i : R * (i + 1), :] = res.results[i]["out"].T
    return y.reshape(B, N, C)


# revision 7
# speedup vs baseline: 1.3725x; 1.3725x over previous
"""Distributed Trainium2 kernel for the dense-transformer attention block:

    xn = LN(x); q,k = xn @ w_qk; v = xn @ w_v
    out = softmax(q k^T / sqrt(hd)) v ; out = LN(out) @ w_proj + b_proj

Sharding: the (B=2, N=2048) token axis is flattened to 4096 rows, 512 per
core (cores 0-3 own batch 0, cores 4-7 batch 1).  Each core runs LN1 and
the K/V projections on its rows, all-gathers K^T and V (fp16) inside its
4-core batch group, computes all 16 heads of attention for its own 512
query rows, then LN2 and the output projection — every FLOP except the
gathers is done exactly once across the chip.

On-chip data lives in a transposed ("feature on partition") layout: the
host feeds x^T per core and transposes the returned y^T back (free).
Matmul operands are fp16 (full PE rate + fast weight load); PSUM
accumulation and LayerNorm statistics stay fp32.  Softmax denominators
come free from a ones-column appended to V (row 64 of the AV output);
no max-subtraction is needed because scores are ~N(0,1).
"""

import numpy as np

import concourse.bass as bass
import concourse.mybir as mybir
import concourse.tile as tile
from concourse import bacc
from concourse.bass_utils import run_bass_kernel_spmd

B, N, C = 2, 2048, 1024
H, HD = 16, 64
NCORES = 8
R = (B * N) // NCORES  # 512 rows per core
GROUP = NCORES // B  # 4 cores per batch
NKC = (GROUP * R) // 128  # 16 k-chunks of 128
EPS = 1e-5
SCALE = HD**-0.5

F32 = mybir.dt.float32
F16 = mybir.dt.float16
AF = mybir.ActivationFunctionType
ALU = mybir.AluOpType

CT = C // 128  # 8 C tiles

last_exec_time_ns = None
_cached_nc = None


def _build():
    nc = bacc.Bacc("TRN2", target_bir_lowering=False, debug=False, num_devices=NCORES)

    xT_ext = nc.dram_tensor("xT", [C, R], F16, kind="ExternalInput")
    wqk_ext = nc.dram_tensor("w_qk", [C, 2 * C], F16, kind="ExternalInput")
    wv_ext = nc.dram_tensor("w_v", [C, C], F16, kind="ExternalInput")
    wp_ext = nc.dram_tensor("w_proj", [C, C], F16, kind="ExternalInput")
    g1_ext = nc.dram_tensor("ln1_g", [C], F32, kind="ExternalInput")
    b1_ext = nc.dram_tensor("ln1_b", [C], F32, kind="ExternalInput")
    g2_ext = nc.dram_tensor("ln2_g", [C], F32, kind="ExternalInput")
    b2_ext = nc.dram_tensor("ln2_b", [C], F32, kind="ExternalInput")
    bp_ext = nc.dram_tensor("b_proj", [C], F32, kind="ExternalInput")
    out_ext = nc.dram_tensor("out", [C, R], F32, kind="ExternalOutput")

    k_in = nc.dram_tensor("k_in", [C, R], F16)  # k^T, feature-major
    k_out = nc.dram_tensor("k_out", [GROUP, C, R], F16)
    v_in = nc.dram_tensor("v_in", [R, H, HD + 1], F16)  # v + ones col per head
    v_out = nc.dram_tensor("v_out", [GROUP, R, H, HD + 1], F16)

    with tile.TileContext(nc) as tc:
        with (
            tc.tile_pool(name="const", bufs=1) as constp,
            tc.tile_pool(name="qT", bufs=1) as qTp,
            tc.tile_pool(name="o", bufs=1) as op_,
        ):
            ones_col = constp.tile([128, 1], F16, tag="onesc")
            nc.vector.memset(ones_col[:], 1.0)
            ones_row = constp.tile([1, 128], F16, tag="onesr")
            nc.vector.memset(ones_row[:], 1.0)
            eps_t = constp.tile([1, 1], F32, tag="epsc")
            nc.sync.dma_start(
                eps_t[:], nc.inline_tensor(np.full((1, 1), EPS, np.float32), "epsc").ap()
            )
            g1 = constp.tile([128, CT], F32, tag="g1")
            nc.sync.dma_start(g1[:], g1_ext.ap().rearrange("(c p) -> p c", p=128))
            b1 = constp.tile([128, CT], F32, tag="b1")
            nc.sync.dma_start(b1[:], b1_ext.ap().rearrange("(c p) -> p c", p=128))
            g2h = constp.tile([64, H], F32, tag="g2h")
            nc.sync.dma_start(g2h[:], g2_ext.ap().rearrange("(h p) -> p h", p=64))
            b2h = constp.tile([64, H], F32, tag="b2h")
            nc.sync.dma_start(b2h[:], b2_ext.ap().rearrange("(h p) -> p h", p=64))
            bp = constp.tile([128, CT], F32, tag="bp")
            nc.sync.dma_start(bp[:], bp_ext.ap().rearrange("(c p) -> p c", p=128))

            qT = [qTp.tile([128, R], F16, tag=f"qT{p}", name=f"qT{p}") for p in range(H // 2)]
            o = [op_.tile([64, R], F16, tag=f"o{h}", name=f"o{h}") for h in range(H)]

            # ---------------- Phase 1: LN1 + K/V/Q projections ----------
            with (
                tc.tile_pool(name="xn", bufs=1) as xnp,
                tc.tile_pool(name="w1", bufs=1) as w1p,
                tc.tile_pool(name="tmp1", bufs=3) as tmp1p,
                tc.tile_pool(name="small1", bufs=1) as small1p,
                tc.tile_pool(name="ev1", bufs=4) as ev1p,
                tc.tile_pool(name="xt", bufs=1) as xtp,
                tc.tile_pool(name="ps_stat", bufs=1, space="PSUM") as ps_stat,
                tc.tile_pool(name="ps_bc", bufs=1, space="PSUM") as ps_bc,
                tc.tile_pool(name="ps_mm", bufs=3, space="PSUM") as ps_mm,
            ):
                xt = []
                for c in range(CT):
                    t = xtp.tile([128, R], F16, tag=f"xt{c}")
                    nc.sync.dma_start(t[:], xT_ext[128 * c : 128 * (c + 1), :])
                    xt.append(t)

                # resident fp16 weights
                wqk_sb = []
                wv_sb = []
                for c in range(CT):
                    t = w1p.tile([128, 2 * C], F16, tag=f"wqk{c}", name=f"wqk{c}")
                    nc.sync.dma_start(t[:], wqk_ext[128 * c : 128 * (c + 1), :])
                    wqk_sb.append(t)
                    t = w1p.tile([128, C], F16, tag=f"wv{c}", name=f"wv{c}")
                    nc.sync.dma_start(t[:], wv_ext[128 * c : 128 * (c + 1), :])
                    wv_sb.append(t)

                # LN1 stats: sums of x and x^2 over C (partition axis)
                sx_ps = ps_stat.tile([1, R], F32, tag="sx")
                sq_ps = ps_stat.tile([1, R], F32, tag="sq")
                for c in range(CT):
                    nc.tensor.matmul(
                        sx_ps[:], ones_col[:], xt[c][:], start=(c == 0), stop=(c == CT - 1)
                    )
                for c in range(CT):
                    xsq = tmp1p.tile([128, R], F16, tag="xsq")
                    nc.scalar.activation(xsq[:], xt[c][:], AF.Square)
                    nc.tensor.matmul(
                        sq_ps[:], ones_col[:], xsq[:], start=(c == 0), stop=(c == CT - 1)
                    )

                mu = small1p.tile([1, R], F16, tag="mu")
                nc.vector.tensor_scalar_mul(mu[:], sx_ps[:], 1.0 / C)
                m2 = small1p.tile([1, R], F32, tag="m2")
                nc.vector.tensor_scalar_mul(m2[:], sq_ps[:], 1.0 / C)
                musq = small1p.tile([1, R], F32, tag="musq")
                nc.vector.tensor_tensor(musq[:], mu[:], mu[:], op=ALU.mult)
                var = small1p.tile([1, R], F32, tag="var")
                nc.vector.tensor_tensor(var[:], m2[:], musq[:], op=ALU.subtract)
                lv = small1p.tile([1, R], F32, tag="lv")
                nc.scalar.activation(lv[:], var[:], AF.Ln, bias=eps_t[:])
                rsig = small1p.tile([1, R], F16, tag="rsig")
                nc.scalar.activation(rsig[:], lv[:], AF.Exp, scale=-0.5)

                bmu_ps = ps_bc.tile([128, R], F32, tag="bmu")
                nc.tensor.matmul(bmu_ps[:], ones_row[:], mu[:], start=True, stop=True)
                brs_ps = ps_bc.tile([128, R], F32, tag="brs")
                nc.tensor.matmul(brs_ps[:], ones_row[:], rsig[:], start=True, stop=True)

                xn = []
                for c in range(CT):
                    t1 = tmp1p.tile([128, R], F32, tag="lt1")
                    nc.vector.tensor_tensor(t1[:], xt[c][:], bmu_ps[:], op=ALU.subtract)
                    t2 = tmp1p.tile([128, R], F32, tag="lt2")
                    nc.vector.tensor_tensor(t2[:], t1[:], brs_ps[:], op=ALU.mult)
                    t3 = xnp.tile([128, R], F16, tag=f"xn{c}")
                    nc.vector.tensor_scalar(
                        t3[:], t2[:], g1[:, c : c + 1], b1[:, c : c + 1],
                        op0=ALU.mult, op1=ALU.add,
                    )
                    xn.append(t3)

                # k^T = w_k^T @ xn^T -> [C, R] feature-major, then gather
                for kf in range(CT):
                    kps = ps_mm.tile([128, R], F32, tag="kvps")
                    for c in range(CT):
                        nc.tensor.matmul(
                            kps[:],
                            wqk_sb[c][:, C + 128 * kf : C + 128 * (kf + 1)],
                            xn[c][:],
                            start=(c == 0),
                            stop=(c == CT - 1),
                        )
                    ksb = ev1p.tile([128, R], F16, tag="kev")
                    nc.scalar.copy(ksb[:], kps[:])
                    nc.sync.dma_start(k_in[128 * kf : 128 * (kf + 1), :], ksb[:])

                nc.gpsimd.collective_compute(
                    "AllGather",
                    ALU.bypass,
                    ins=[k_in[:]],
                    outs=[k_out[:]],
                    replica_groups=[[0, 1, 2, 3], [4, 5, 6, 7]],
                )

                # v = xn @ w_v -> [R, H, 65] row-major with ones cols, gather
                for rt in range(R // 128):
                    for vf in range(2):
                        vps = ps_mm.tile([128, R], F32, tag="kvps")
                        for c in range(CT):
                            nc.tensor.matmul(
                                vps[:],
                                xn[c][:, 128 * rt : 128 * (rt + 1)],
                                wv_sb[c][:, 512 * vf : 512 * (vf + 1)],
                                start=(c == 0),
                                stop=(c == CT - 1),
                            )
                        vsb = ev1p.tile([128, 8, HD + 1], F16, tag="vev")
                        nc.scalar.copy(
                            vsb[:, :, 0:HD],
                            vps[:].rearrange("p (h d) -> p h d", d=HD),
                        )
                        nc.vector.memset(vsb[:, :, HD : HD + 1], 1.0)
                        nc.sync.dma_start(
                            v_in[128 * rt : 128 * (rt + 1), 8 * vf : 8 * (vf + 1), :],
                            vsb[:],
                        )

                nc.gpsimd.collective_compute(
                    "AllGather",
                    ALU.bypass,
                    ins=[v_in[:]],
                    outs=[v_out[:]],
                    replica_groups=[[0, 1, 2, 3], [4, 5, 6, 7]],
                )

                # q^T = w_q^T @ xn^T -> [C, R] (overlaps the all-gathers)
                for qf in range(CT):
                    qps = ps_mm.tile([128, R], F32, tag="kvps")
                    for c in range(CT):
                        nc.tensor.matmul(
                            qps[:],
                            wqk_sb[c][:, 128 * qf : 128 * (qf + 1)],
                            xn[c][:],
                            start=(c == 0),
                            stop=(c == CT - 1),
                        )
                    nc.scalar.copy(qT[qf][:], qps[:])

            # ---------------- Phase 2: attention, software-pipelined pairs
            with (
                tc.tile_pool(name="kpair", bufs=3) as kpp,
                tc.tile_pool(name="vpair", bufs=3) as vpp,
                tc.tile_pool(name="att", bufs=5) as attp,
                tc.tile_pool(name="rec", bufs=4) as recp,
                tc.tile_pool(name="ps_sc", bufs=2, space="PSUM") as ps_sc,
                tc.tile_pool(name="ps_av", bufs=2, space="PSUM") as ps_av,
                tc.tile_pool(name="ps_nb", bufs=2, space="PSUM") as ps_nb,
            ):
                NP = H // 2
                kpair = [None] * NP
                vpair = [None] * NP
                att = [None] * NP

                def load_pair(p):
                    kpair[p] = kpp.tile([128, GROUP * R], F16, tag="kpair", name=f"kp{p}")
                    nc.sync.dma_start(
                        kpair[p][:].rearrange("p (r n) -> p r n", r=GROUP),
                        k_out[:, 128 * p : 128 * (p + 1), :].rearrange("r p n -> p r n"),
                    )
                    vpair[p] = vpp.tile(
                        [128, NKC, 2 * (HD + 1)], F16, tag="vpair", name=f"vp{p}"
                    )
                    nc.sync.dma_start(
                        vpair[p][:],
                        v_out[:]
                        .rearrange("r (j p) h d -> p (r j) (h d)", p=128)[
                            :, :, (HD + 1) * 2 * p : (HD + 1) * 2 * (p + 1)
                        ],
                    )

                def scores_pair(p):
                    att[p] = [[], []]
                    for g in range(NKC // 2):
                        scp = [None, None]
                        for hi in range(2):
                            base = 64 * hi
                            scp[hi] = ps_sc.tile(
                                [128, 1024], F32, tag="scps", name=f"sc{p}_{g}_{hi}"
                            )
                            for cc in range(2):
                                j = 2 * g + cc
                                nc.tensor.matmul(
                                    scp[hi][:, 512 * cc : 512 * (cc + 1)],
                                    kpair[p][base : base + 64, 128 * j : 128 * (j + 1)],
                                    qT[p][base : base + 64, :],
                                    start=True,
                                    stop=True,
                                )
                        for hi in range(2):
                            at = attp.tile([128, 1024], F16, tag="att")
                            nc.scalar.activation(at[:], scp[hi][:], AF.Exp, scale=SCALE)
                            att[p][hi].append(at)

                def av_pair(p):
                    for hi in range(2):
                        h = 2 * p + hi
                        avps = ps_av.tile([HD + 1, R], F32, tag="avps", name=f"av{h}")
                        for j in range(NKC):
                            nc.tensor.matmul(
                                avps[:],
                                vpair[p][:, j, (HD + 1) * hi : (HD + 1) * (hi + 1)],
                                att[p][hi][j // 2][:, 512 * (j % 2) : 512 * (j % 2 + 1)],
                                start=(j == 0),
                                stop=(j == NKC - 1),
                            )
                        rec = recp.tile([1, R], F16, tag="rec")
                        with nc.allow_low_precision(reason="softmax denom bcast"):
                            nc.vector.reciprocal(rec[:], avps[HD : HD + 1, :])
                        nbps = ps_nb.tile([64, R], F32, tag="nbps", name=f"nb{h}")
                        nc.tensor.matmul(
                            nbps[:], ones_row[:, 0:64], rec[:], start=True, stop=True
                        )
                        nc.vector.tensor_copy(o[h][:], avps[0:HD, :])
                        nc.vector.tensor_tensor(o[h][:], o[h][:], nbps[:], op=ALU.mult)
                    att[p] = None

                load_pair(0)
                scores_pair(0)
                for p in range(NP):
                    if p + 1 < NP:
                        load_pair(p + 1)
                        scores_pair(p + 1)
                    av_pair(p)

            # ---------------- Phase 3: LN2 + projection ------------------
            with (
                tc.tile_pool(name="wp", bufs=1) as wpp,
                tc.tile_pool(name="tmp2", bufs=4) as tmp2p,
                tc.tile_pool(name="small2", bufs=1) as small2p,
                tc.tile_pool(name="ln2o", bufs=1) as ln2op,
                tc.tile_pool(name="yev", bufs=3) as yevp,
                tc.tile_pool(name="ps_stat2", bufs=1, space="PSUM") as ps_stat2,
                tc.tile_pool(name="ps_bc2", bufs=1, space="PSUM") as ps_bc2,
                tc.tile_pool(name="ps_y", bufs=3, space="PSUM") as ps_y,
            ):
                wp_sb = []
                for h in range(H):
                    t = wpp.tile([64, C], F16, tag=f"wp{h}", name=f"wp{h}")
                    nc.sync.dma_start(t[:], wp_ext[HD * h : HD * (h + 1), :])
                    wp_sb.append(t)

                s2_ps = ps_stat2.tile([1, R], F32, tag="s2")
                q2_ps = ps_stat2.tile([1, R], F32, tag="q2")
                for h in range(H):
                    nc.tensor.matmul(
                        s2_ps[:], ones_col[0:64, :], o[h][:],
                        start=(h == 0), stop=(h == H - 1),
                    )
                for h in range(H):
                    osq = tmp2p.tile([64, R], F16, tag="osq")
                    nc.scalar.activation(osq[:], o[h][:], AF.Square)
                    nc.tensor.matmul(
                        q2_ps[:], ones_col[0:64, :], osq[:],
                        start=(h == 0), stop=(h == H - 1),
                    )

                mu2 = small2p.tile([1, R], F16, tag="mu2")
                nc.vector.tensor_scalar_mul(mu2[:], s2_ps[:], 1.0 / C)
                m22 = small2p.tile([1, R], F32, tag="m22")
                nc.vector.tensor_scalar_mul(m22[:], q2_ps[:], 1.0 / C)
                musq2 = small2p.tile([1, R], F32, tag="musq2")
                nc.vector.tensor_tensor(musq2[:], mu2[:], mu2[:], op=ALU.mult)
                var2 = small2p.tile([1, R], F32, tag="var2")
                nc.vector.tensor_tensor(var2[:], m22[:], musq2[:], op=ALU.subtract)
                lv2 = small2p.tile([1, R], F32, tag="lv2")
                nc.scalar.activation(lv2[:], var2[:], AF.Ln, bias=eps_t[:])
                rsig2 = small2p.tile([1, R], F16, tag="rsig2")
                nc.scalar.activation(rsig2[:], lv2[:], AF.Exp, scale=-0.5)

                bmu2_ps = ps_bc2.tile([64, R], F32, tag="bmu2")
                nc.tensor.matmul(
                    bmu2_ps[:], ones_row[:, 0:64], mu2[:], start=True, stop=True
                )
                brs2_ps = ps_bc2.tile([64, R], F32, tag="brs2")
                nc.tensor.matmul(
                    brs2_ps[:], ones_row[:, 0:64], rsig2[:], start=True, stop=True
                )

                ln2o = []
                for h in range(H):
                    t1 = tmp2p.tile([64, R], F32, tag="l2t1")
                    nc.vector.tensor_tensor(t1[:], o[h][:], bmu2_ps[:], op=ALU.subtract)
                    t2 = tmp2p.tile([64, R], F32, tag="l2t2")
                    nc.vector.tensor_tensor(t2[:], t1[:], brs2_ps[:], op=ALU.mult)
                    t3 = ln2op.tile([64, R], F16, tag=f"ln2o{h}", name=f"ln2o{h}")
                    nc.vector.tensor_scalar(
                        t3[:], t2[:], g2h[:, h : h + 1], b2h[:, h : h + 1],
                        op0=ALU.mult, op1=ALU.add,
                    )
                    ln2o.append(t3)

                for of in range(CT):
                    yps = ps_y.tile([128, R], F32, tag="yps")
                    for h in range(H):
                        nc.tensor.matmul(
                            yps[:],
                            wp_sb[h][:, 128 * of : 128 * (of + 1)],
                            ln2o[h][:],
                            start=(h == 0),
                            stop=(h == H - 1),
                        )
                    ysb = yevp.tile([128, R], F32, tag="yev")
                    nc.scalar.activation(
                        ysb[:], yps[:], AF.Identity, bias=bp[:, of : of + 1]
                    )
                    nc.sync.dma_start(out_ext[128 * of : 128 * (of + 1), :], ysb[:])

    nc.compile()
    return nc


def kernel(x, ln1_g, ln1_b, w_qk, w_v, ln2_g, ln2_b, w_proj, b_proj):
    global _cached_nc, last_exec_time_ns
    if _cached_nc is None:
        _cached_nc = _build()
    nc = _cached_nc

    xr = np.asarray(x, np.float32).reshape(B * N, C).astype(np.float16)
    shared = {
        "w_qk": np.asarray(w_qk, np.float32).astype(np.float16),
        "w_v": np.asarray(w_v, np.float32).astype(np.float16),
        "w_proj": np.asarray(w_proj, np.float32).astype(np.float16),
        "ln1_g": np.ascontiguousarray(ln1_g, np.float32),
        "ln1_b": np.ascontiguousarray(ln1_b, np.float32),
        "ln2_g": np.ascontiguousarray(ln2_g, np.float32),
        "ln2_b": np.ascontiguousarray(ln2_b, np.float32),
        "b_proj": np.ascontiguousarray(b_proj, np.float32),
    }
    in_maps = []
    for i in range(NCORES):
        xT_i = np.ascontiguousarray(xr[R * i : R * (i + 1), :].T)
        in_maps.append({"xT": xT_i, **shared})

    res = run_bass_kernel_spmd(nc, in_maps, core_ids=list(range(NCORES)))
    last_exec_time_ns = res.exec_time_ns

    y = np.empty((B * N, C), np.float32)
    for i in range(NCORES):
        y[R * i : R * (i + 1), :] = res.results[i]["out"].T
    return y.reshape(B, N, C)


# revision 9
# speedup vs baseline: 1.5589x; 1.1358x over previous
"""Distributed Trainium2 kernel for the dense-transformer attention block:

    xn = LN(x); q,k = xn @ w_qk; v = xn @ w_v
    out = softmax(q k^T / sqrt(hd)) v ; out = LN(out) @ w_proj + b_proj

Sharding: the (B=2, N=2048) token axis is flattened to 4096 rows, 512 per
core (cores 0-3 own batch 0, cores 4-7 batch 1).  Each core runs LN1 and
the K/V projections on its rows, all-gathers K^T and V (fp16) inside its
4-core batch group, computes all 16 heads of attention for its own 512
query rows, then LN2 and the output projection — every FLOP except the
gathers is done exactly once across the chip.

On-chip data lives in a transposed ("feature on partition") layout: the
host feeds x^T per core and transposes the returned y^T back (free).
Matmul operands are fp16 (full PE rate + fast weight load); PSUM
accumulation and LayerNorm statistics stay fp32.  Softmax denominators
come free from a ones-column appended to V (row 64 of the AV output);
no max-subtraction is needed because scores are ~N(0,1).
"""

import numpy as np

import concourse.bass as bass
import concourse.mybir as mybir
import concourse.tile as tile
from concourse import bacc
from concourse.bass_utils import run_bass_kernel_spmd

B, N, C = 2, 2048, 1024
H, HD = 16, 64
NCORES = 8
R = (B * N) // NCORES  # 512 rows per core
GROUP = NCORES // B  # 4 cores per batch
NKC = (GROUP * R) // 128  # 16 k-chunks of 128
EPS = 1e-5
SCALE = HD**-0.5

F32 = mybir.dt.float32
F16 = mybir.dt.float16
AF = mybir.ActivationFunctionType
ALU = mybir.AluOpType

CT = C // 128  # 8 C tiles

last_exec_time_ns = None
_cached_nc = None


def _build():
    nc = bacc.Bacc("TRN2", target_bir_lowering=False, debug=False, num_devices=NCORES)

    xT_ext = nc.dram_tensor("xT", [C, R], F16, kind="ExternalInput")
    wqk_ext = nc.dram_tensor("w_qk", [C, 2 * C], F16, kind="ExternalInput")
    wv_ext = nc.dram_tensor("w_v", [C, C], F16, kind="ExternalInput")
    wp_ext = nc.dram_tensor("w_proj", [C, C], F16, kind="ExternalInput")
    g1_ext = nc.dram_tensor("ln1_g", [C], F32, kind="ExternalInput")
    b1_ext = nc.dram_tensor("ln1_b", [C], F32, kind="ExternalInput")
    g2_ext = nc.dram_tensor("ln2_g", [C], F32, kind="ExternalInput")
    b2_ext = nc.dram_tensor("ln2_b", [C], F32, kind="ExternalInput")
    bp_ext = nc.dram_tensor("b_proj", [C], F32, kind="ExternalInput")
    out_ext = nc.dram_tensor("out", [C, R], F32, kind="ExternalOutput")

    k_in = nc.dram_tensor("k_in", [C, R], F16)  # k^T, feature-major
    k_out = nc.dram_tensor("k_out", [GROUP, C, R], F16)
    v_in = nc.dram_tensor("v_in", [R, H, HD + 1], F16)  # v + ones col per head
    v_out = nc.dram_tensor("v_out", [GROUP, R, H, HD + 1], F16)

    with tile.TileContext(nc) as tc:
        with (
            tc.tile_pool(name="const", bufs=1) as constp,
            tc.tile_pool(name="qT", bufs=1) as qTp,
            tc.tile_pool(name="o", bufs=1) as op_,
        ):
            ones_col = constp.tile([128, 1], F16, tag="onesc")
            nc.vector.memset(ones_col[:], 1.0)
            ones_row = constp.tile([1, 128], F16, tag="onesr")
            nc.vector.memset(ones_row[:], 1.0)
            eps_t = constp.tile([1, 1], F32, tag="epsc")
            nc.sync.dma_start(
                eps_t[:], nc.inline_tensor(np.full((1, 1), EPS, np.float32), "epsc").ap()
            )
            g1 = constp.tile([128, CT], F32, tag="g1")
            nc.sync.dma_start(g1[:], g1_ext.ap().rearrange("(c p) -> p c", p=128))
            b1 = constp.tile([128, CT], F32, tag="b1")
            nc.sync.dma_start(b1[:], b1_ext.ap().rearrange("(c p) -> p c", p=128))
            g2h = constp.tile([64, H], F32, tag="g2h")
            nc.sync.dma_start(g2h[:], g2_ext.ap().rearrange("(h p) -> p h", p=64))
            b2h = constp.tile([64, H], F32, tag="b2h")
            nc.sync.dma_start(b2h[:], b2_ext.ap().rearrange("(h p) -> p h", p=64))
            bp = constp.tile([128, CT], F32, tag="bp")
            nc.sync.dma_start(bp[:], bp_ext.ap().rearrange("(c p) -> p c", p=128))

            qT = [qTp.tile([128, R], F16, tag=f"qT{p}", name=f"qT{p}") for p in range(H // 2)]
            o = [op_.tile([64, R], F16, tag=f"o{h}", name=f"o{h}") for h in range(H)]

            # ---------------- Phase 1: LN1 + K/V/Q projections ----------
            with (
                tc.tile_pool(name="xn", bufs=1) as xnp,
                tc.tile_pool(name="w1", bufs=1) as w1p,
                tc.tile_pool(name="tmp1", bufs=3) as tmp1p,
                tc.tile_pool(name="small1", bufs=1) as small1p,
                tc.tile_pool(name="ev1", bufs=4) as ev1p,
                tc.tile_pool(name="xt", bufs=1) as xtp,
                tc.tile_pool(name="ps_stat", bufs=1, space="PSUM") as ps_stat,
                tc.tile_pool(name="ps_bc", bufs=1, space="PSUM") as ps_bc,
                tc.tile_pool(name="ps_mm", bufs=3, space="PSUM") as ps_mm,
            ):
                xt = []
                for c in range(CT):
                    t = xtp.tile([128, R], F16, tag=f"xt{c}")
                    nc.sync.dma_start(t[:], xT_ext[128 * c : 128 * (c + 1), :])
                    xt.append(t)

                # resident fp16 weights
                wqk_sb = []
                wv_sb = []
                for c in range(CT):
                    t = w1p.tile([128, 2 * C], F16, tag=f"wqk{c}", name=f"wqk{c}")
                    nc.sync.dma_start(t[:], wqk_ext[128 * c : 128 * (c + 1), :])
                    wqk_sb.append(t)
                    t = w1p.tile([128, C], F16, tag=f"wv{c}", name=f"wv{c}")
                    nc.sync.dma_start(t[:], wv_ext[128 * c : 128 * (c + 1), :])
                    wv_sb.append(t)

                # LN1 stats: sums of x and x^2 over C (partition axis)
                sx_ps = ps_stat.tile([1, R], F32, tag="sx")
                sq_ps = ps_stat.tile([1, R], F32, tag="sq")
                for c in range(CT):
                    nc.tensor.matmul(
                        sx_ps[:], ones_col[:], xt[c][:], start=(c == 0), stop=(c == CT - 1)
                    )
                for c in range(CT):
                    xsq = tmp1p.tile([128, R], F16, tag="xsq")
                    nc.scalar.activation(xsq[:], xt[c][:], AF.Square)
                    nc.tensor.matmul(
                        sq_ps[:], ones_col[:], xsq[:], start=(c == 0), stop=(c == CT - 1)
                    )

                mu = small1p.tile([1, R], F16, tag="mu")
                nc.vector.tensor_scalar_mul(mu[:], sx_ps[:], 1.0 / C)
                m2 = small1p.tile([1, R], F32, tag="m2")
                nc.vector.tensor_scalar_mul(m2[:], sq_ps[:], 1.0 / C)
                musq = small1p.tile([1, R], F32, tag="musq")
                nc.vector.tensor_tensor(musq[:], mu[:], mu[:], op=ALU.mult)
                var = small1p.tile([1, R], F32, tag="var")
                nc.vector.tensor_tensor(var[:], m2[:], musq[:], op=ALU.subtract)
                lv = small1p.tile([1, R], F32, tag="lv")
                nc.scalar.activation(lv[:], var[:], AF.Ln, bias=eps_t[:])
                rsig = small1p.tile([1, R], F16, tag="rsig")
                nc.scalar.activation(rsig[:], lv[:], AF.Exp, scale=-0.5)

                bmu_ps = ps_bc.tile([128, R], F32, tag="bmu")
                nc.tensor.matmul(bmu_ps[:], ones_row[:], mu[:], start=True, stop=True)
                brs_ps = ps_bc.tile([128, R], F32, tag="brs")
                nc.tensor.matmul(brs_ps[:], ones_row[:], rsig[:], start=True, stop=True)

                xn = []
                for c in range(CT):
                    t1 = tmp1p.tile([128, R], F32, tag="lt1")
                    nc.vector.tensor_tensor(t1[:], xt[c][:], bmu_ps[:], op=ALU.subtract)
                    t2 = tmp1p.tile([128, R], F32, tag="lt2")
                    nc.vector.tensor_tensor(t2[:], t1[:], brs_ps[:], op=ALU.mult)
                    t3 = xnp.tile([128, R], F16, tag=f"xn{c}")
                    nc.vector.tensor_scalar(
                        t3[:], t2[:], g1[:, c : c + 1], b1[:, c : c + 1],
                        op0=ALU.mult, op1=ALU.add,
                    )
                    xn.append(t3)

                # k^T = w_k^T @ xn^T -> [C, R] feature-major, then gather
                for kf in range(CT):
                    kps = ps_mm.tile([128, R], F32, tag="kvps")
                    for c in range(CT):
                        nc.tensor.matmul(
                            kps[:],
                            wqk_sb[c][:, C + 128 * kf : C + 128 * (kf + 1)],
                            xn[c][:],
                            start=(c == 0),
                            stop=(c == CT - 1),
                        )
                    ksb = ev1p.tile([128, R], F16, tag="kev")
                    nc.scalar.copy(ksb[:], kps[:])
                    nc.sync.dma_start(k_in[128 * kf : 128 * (kf + 1), :], ksb[:])

                nc.gpsimd.collective_compute(
                    "AllGather",
                    ALU.bypass,
                    ins=[k_in[:]],
                    outs=[k_out[:]],
                    replica_groups=[[0, 1, 2, 3], [4, 5, 6, 7]],
                )

                # v = xn @ w_v -> [R, H, 65] row-major with ones cols, gather
                for rt in range(R // 128):
                    for vf in range(2):
                        vps = ps_mm.tile([128, R], F32, tag="kvps")
                        for c in range(CT):
                            nc.tensor.matmul(
                                vps[:],
                                xn[c][:, 128 * rt : 128 * (rt + 1)],
                                wv_sb[c][:, 512 * vf : 512 * (vf + 1)],
                                start=(c == 0),
                                stop=(c == CT - 1),
                            )
                        vsb = ev1p.tile([128, 8, HD + 1], F16, tag="vev")
                        nc.scalar.copy(
                            vsb[:, :, 0:HD],
                            vps[:].rearrange("p (h d) -> p h d", d=HD),
                        )
                        nc.vector.memset(vsb[:, :, HD : HD + 1], 1.0)
                        nc.sync.dma_start(
                            v_in[128 * rt : 128 * (rt + 1), 8 * vf : 8 * (vf + 1), :],
                            vsb[:],
                        )

                nc.gpsimd.collective_compute(
                    "AllGather",
                    ALU.bypass,
                    ins=[v_in[:]],
                    outs=[v_out[:]],
                    replica_groups=[[0, 1, 2, 3], [4, 5, 6, 7]],
                )

                # q^T = w_q^T @ xn^T -> [C, R] (overlaps the all-gathers)
                for qf in range(CT):
                    qps = ps_mm.tile([128, R], F32, tag="kvps")
                    for c in range(CT):
                        nc.tensor.matmul(
                            qps[:],
                            wqk_sb[c][:, 128 * qf : 128 * (qf + 1)],
                            xn[c][:],
                            start=(c == 0),
                            stop=(c == CT - 1),
                        )
                    nc.scalar.copy(qT[qf][:], qps[:])

            # ---------------- Phase 2: attention -------------------------
            # All score/exp work first (depends only on the K gather), then
            # all AV work (needs the V gather) — keeps the PE busy through
            # the V all-gather window.  att slots throttle the lookahead.
            with (
                tc.tile_pool(name="kpair", bufs=2) as kpp,
                tc.tile_pool(name="vpair", bufs=3) as vpp,
                tc.tile_pool(name="att", bufs=48) as attp,
                tc.tile_pool(name="rec", bufs=4) as recp,
                tc.tile_pool(name="ps_sc", bufs=2, space="PSUM") as ps_sc,
                tc.tile_pool(name="ps_av", bufs=2, space="PSUM") as ps_av,
                tc.tile_pool(name="ps_nb", bufs=2, space="PSUM") as ps_nb,
            ):
                NP = H // 2
                att = [None] * NP

                for p in range(NP):
                    kpair = kpp.tile([128, GROUP * R], F16, tag="kpair", name=f"kp{p}")
                    nc.sync.dma_start(
                        kpair[:].rearrange("p (r n) -> p r n", r=GROUP),
                        k_out[:, 128 * p : 128 * (p + 1), :].rearrange("r p n -> p r n"),
                    )
                    att[p] = [[], []]
                    for g in range(NKC // 2):
                        scp = [None, None]
                        for hi in range(2):
                            base = 64 * hi
                            scp[hi] = ps_sc.tile(
                                [128, 1024], F32, tag="scps", name=f"sc{p}_{g}_{hi}"
                            )
                            for cc in range(2):
                                j = 2 * g + cc
                                nc.tensor.matmul(
                                    scp[hi][:, 512 * cc : 512 * (cc + 1)],
                                    kpair[base : base + 64, 128 * j : 128 * (j + 1)],
                                    qT[p][base : base + 64, :],
                                    start=True,
                                    stop=True,
                                )
                        for hi in range(2):
                            at = attp.tile([128, 1024], F16, tag="att")
                            nc.scalar.activation(at[:], scp[hi][:], AF.Exp, scale=SCALE)
                            att[p][hi].append(at)

                for p in range(NP):
                    vpair = vpp.tile(
                        [128, NKC, 2 * (HD + 1)], F16, tag="vpair", name=f"vp{p}"
                    )
                    nc.sync.dma_start(
                        vpair[:],
                        v_out[:]
                        .rearrange("r (j p) h d -> p (r j) (h d)", p=128)[
                            :, :, (HD + 1) * 2 * p : (HD + 1) * 2 * (p + 1)
                        ],
                    )
                    for hi in range(2):
                        h = 2 * p + hi
                        avps = ps_av.tile([HD + 1, R], F32, tag="avps", name=f"av{h}")
                        for j in range(NKC):
                            nc.tensor.matmul(
                                avps[:],
                                vpair[:, j, (HD + 1) * hi : (HD + 1) * (hi + 1)],
                                att[p][hi][j // 2][:, 512 * (j % 2) : 512 * (j % 2 + 1)],
                                start=(j == 0),
                                stop=(j == NKC - 1),
                            )
                        rec = recp.tile([1, R], F16, tag="rec")
                        with nc.allow_low_precision(reason="softmax denom bcast"):
                            nc.vector.reciprocal(rec[:], avps[HD : HD + 1, :])
                        nbps = ps_nb.tile([64, R], F32, tag="nbps", name=f"nb{h}")
                        nc.tensor.matmul(
                            nbps[:], ones_row[:, 0:64], rec[:], start=True, stop=True
                        )
                        nc.vector.tensor_copy(o[h][:], avps[0:HD, :])
                        nc.vector.tensor_tensor(o[h][:], o[h][:], nbps[:], op=ALU.mult)
                    att[p] = None

            # ---------------- Phase 3: LN2 + projection ------------------
            with (
                tc.tile_pool(name="wp", bufs=1) as wpp,
                tc.tile_pool(name="tmp2", bufs=4) as tmp2p,
                tc.tile_pool(name="small2", bufs=1) as small2p,
                tc.tile_pool(name="ln2o", bufs=1) as ln2op,
                tc.tile_pool(name="yev", bufs=3) as yevp,
                tc.tile_pool(name="ps_stat2", bufs=1, space="PSUM") as ps_stat2,
                tc.tile_pool(name="ps_bc2", bufs=1, space="PSUM") as ps_bc2,
                tc.tile_pool(name="ps_y", bufs=3, space="PSUM") as ps_y,
            ):
                wp_sb = []
                for h in range(H):
                    t = wpp.tile([64, C], F16, tag=f"wp{h}", name=f"wp{h}")
                    nc.sync.dma_start(t[:], wp_ext[HD * h : HD * (h + 1), :])
                    wp_sb.append(t)

                s2_ps = ps_stat2.tile([1, R], F32, tag="s2")
                q2_ps = ps_stat2.tile([1, R], F32, tag="q2")
                for h in range(H):
                    nc.tensor.matmul(
                        s2_ps[:], ones_col[0:64, :], o[h][:],
                        start=(h == 0), stop=(h == H - 1),
                    )
                for h in range(H):
                    osq = tmp2p.tile([64, R], F16, tag="osq")
                    nc.scalar.activation(osq[:], o[h][:], AF.Square)
                    nc.tensor.matmul(
                        q2_ps[:], ones_col[0:64, :], osq[:],
                        start=(h == 0), stop=(h == H - 1),
                    )

                mu2 = small2p.tile([1, R], F16, tag="mu2")
                nc.vector.tensor_scalar_mul(mu2[:], s2_ps[:], 1.0 / C)
                m22 = small2p.tile([1, R], F32, tag="m22")
                nc.vector.tensor_scalar_mul(m22[:], q2_ps[:], 1.0 / C)
                musq2 = small2p.tile([1, R], F32, tag="musq2")
                nc.vector.tensor_tensor(musq2[:], mu2[:], mu2[:], op=ALU.mult)
                var2 = small2p.tile([1, R], F32, tag="var2")
                nc.vector.tensor_tensor(var2[:], m22[:], musq2[:], op=ALU.subtract)
                lv2 = small2p.tile([1, R], F32, tag="lv2")
                nc.scalar.activation(lv2[:], var2[:], AF.Ln, bias=eps_t[:])
                rsig2 = small2p.tile([1, R], F16, tag="rsig2")
                nc.scalar.activation(rsig2[:], lv2[:], AF.Exp, scale=-0.5)

                bmu2_ps = ps_bc2.tile([64, R], F32, tag="bmu2")
                nc.tensor.matmul(
                    bmu2_ps[:], ones_row[:, 0:64], mu2[:], start=True, stop=True
                )
                brs2_ps = ps_bc2.tile([64, R], F32, tag="brs2")
                nc.tensor.matmul(
                    brs2_ps[:], ones_row[:, 0:64], rsig2[:], start=True, stop=True
                )

                ln2o = []
                for h in range(H):
                    t1 = tmp2p.tile([64, R], F32, tag="l2t1")
                    nc.vector.tensor_tensor(t1[:], o[h][:], bmu2_ps[:], op=ALU.subtract)
                    t2 = tmp2p.tile([64, R], F32, tag="l2t2")
                    nc.vector.tensor_tensor(t2[:], t1[:], brs2_ps[:], op=ALU.mult)
                    t3 = ln2op.tile([64, R], F16, tag=f"ln2o{h}", name=f"ln2o{h}")
                    nc.vector.tensor_scalar(
                        t3[:], t2[:], g2h[:, h : h + 1], b2h[:, h : h + 1],
                        op0=ALU.mult, op1=ALU.add,
                    )
                    ln2o.append(t3)

                for of in range(CT):
                    yps = ps_y.tile([128, R], F32, tag="yps")
                    for h in range(H):
                        nc.tensor.matmul(
                            yps[:],
                            wp_sb[h][:, 128 * of : 128 * (of + 1)],
                            ln2o[h][:],
                            start=(h == 0),
                            stop=(h == H - 1),
                        )
                    ysb = yevp.tile([128, R], F32, tag="yev")
                    nc.scalar.activation(
                        ysb[:], yps[:], AF.Identity, bias=bp[:, of : of + 1]
                    )
                    nc.sync.dma_start(out_ext[128 * of : 128 * (of + 1), :], ysb[:])

    nc.compile()
    return nc


def kernel(x, ln1_g, ln1_b, w_qk, w_v, ln2_g, ln2_b, w_proj, b_proj):
    global _cached_nc, last_exec_time_ns
    if _cached_nc is None:
        _cached_nc = _build()
    nc = _cached_nc

    xr = np.asarray(x, np.float32).reshape(B * N, C).astype(np.float16)
    shared = {
        "w_qk": np.asarray(w_qk, np.float32).astype(np.float16),
        "w_v": np.asarray(w_v, np.float32).astype(np.float16),
        "w_proj": np.asarray(w_proj, np.float32).astype(np.float16),
        "ln1_g": np.ascontiguousarray(ln1_g, np.float32),
        "ln1_b": np.ascontiguousarray(ln1_b, np.float32),
        "ln2_g": np.ascontiguousarray(ln2_g, np.float32),
        "ln2_b": np.ascontiguousarray(ln2_b, np.float32),
        "b_proj": np.ascontiguousarray(b_proj, np.float32),
    }
    in_maps = []
    for i in range(NCORES):
        xT_i = np.ascontiguousarray(xr[R * i : R * (i + 1), :].T)
        in_maps.append({"xT": xT_i, **shared})

    res = run_bass_kernel_spmd(nc, in_maps, core_ids=list(range(NCORES)))
    last_exec_time_ns = res.exec_time_ns

    y = np.empty((B * N, C), np.float32)
    for i in range(NCORES):
        y[R * i : R * (i + 1), :] = res.results[i]["out"].T
    return y.reshape(B, N, C)


# revision 10
# speedup vs baseline: 1.7409x; 1.1168x over previous
"""Distributed Trainium2 kernel for the dense-transformer attention block:

    xn = LN(x); q,k = xn @ w_qk; v = xn @ w_v
    out = softmax(q k^T / sqrt(hd)) v ; out = LN(out) @ w_proj + b_proj

Sharding: the (B=2, N=2048) token axis is flattened to 4096 rows, 512 per
core (cores 0-3 own batch 0, cores 4-7 batch 1).  Each core runs LN1 and
the K/V projections on its rows, all-gathers K^T and V (fp16) inside its
4-core batch group, computes all 16 heads of attention for its own 512
query rows, then LN2 and the output projection — every FLOP except the
gathers is done exactly once across the chip.

On-chip data lives in a transposed ("feature on partition") layout: the
host feeds x^T per core and transposes the returned y^T back (free).
Matmul operands are fp16 (full PE rate + fast weight load); PSUM
accumulation and LayerNorm statistics stay fp32.  Softmax denominators
come free from a ones-column appended to V (row 64 of the AV output);
no max-subtraction is needed because scores are ~N(0,1).
"""

import numpy as np

import concourse.bass as bass
import concourse.mybir as mybir
import concourse.tile as tile
from concourse import bacc
from concourse.bass_utils import run_bass_kernel_spmd

B, N, C = 2, 2048, 1024
H, HD = 16, 64
NCORES = 8
R = (B * N) // NCORES  # 512 rows per core
GROUP = NCORES // B  # 4 cores per batch
NKC = (GROUP * R) // 128  # 16 k-chunks of 128
EPS = 1e-5
SCALE = HD**-0.5

F32 = mybir.dt.float32
F16 = mybir.dt.float16
AF = mybir.ActivationFunctionType
ALU = mybir.AluOpType

CT = C // 128  # 8 C tiles

last_exec_time_ns = None
_cached_nc = None


def _build():
    nc = bacc.Bacc("TRN2", target_bir_lowering=False, debug=False, num_devices=NCORES)

    xT_ext = nc.dram_tensor("xT", [C, R], F16, kind="ExternalInput")
    wqk_ext = nc.dram_tensor("w_qk", [C, 2 * C], F16, kind="ExternalInput")
    wv_ext = nc.dram_tensor("w_v", [C, C], F16, kind="ExternalInput")
    wp_ext = nc.dram_tensor("w_proj", [C, C], F16, kind="ExternalInput")
    g1_ext = nc.dram_tensor("ln1_g", [C], F32, kind="ExternalInput")
    b1_ext = nc.dram_tensor("ln1_b", [C], F32, kind="ExternalInput")
    g2_ext = nc.dram_tensor("ln2_g", [C], F32, kind="ExternalInput")
    b2_ext = nc.dram_tensor("ln2_b", [C], F32, kind="ExternalInput")
    bp_ext = nc.dram_tensor("b_proj", [C], F32, kind="ExternalInput")
    out_ext = nc.dram_tensor("out", [C, R], F32, kind="ExternalOutput")

    k_in = nc.dram_tensor("k_in", [C, R], F16)  # k^T, feature-major
    k_out = nc.dram_tensor("k_out", [GROUP, C, R], F16)
    v_in = nc.dram_tensor("v_in", [R, H, HD + 1], F16)  # v + ones col per head
    v_out = nc.dram_tensor("v_out", [GROUP, R, H, HD + 1], F16)

    with tile.TileContext(nc) as tc:
        with (
            tc.tile_pool(name="const", bufs=1) as constp,
            tc.tile_pool(name="qT", bufs=1) as qTp,
            tc.tile_pool(name="o", bufs=1) as op_,
        ):
            ones_col = constp.tile([128, 1], F16, tag="onesc")
            nc.vector.memset(ones_col[:], 1.0)
            ones_row = constp.tile([1, 128], F16, tag="onesr")
            nc.vector.memset(ones_row[:], 1.0)
            eps_t = constp.tile([1, 1], F32, tag="epsc")
            nc.sync.dma_start(
                eps_t[:], nc.inline_tensor(np.full((1, 1), EPS, np.float32), "epsc").ap()
            )
            g1 = constp.tile([128, CT], F32, tag="g1")
            nc.sync.dma_start(g1[:], g1_ext.ap().rearrange("(c p) -> p c", p=128))
            b1 = constp.tile([128, CT], F32, tag="b1")
            nc.sync.dma_start(b1[:], b1_ext.ap().rearrange("(c p) -> p c", p=128))
            g2h = constp.tile([64, H], F32, tag="g2h")
            nc.sync.dma_start(g2h[:], g2_ext.ap().rearrange("(h p) -> p h", p=64))
            b2h = constp.tile([64, H], F32, tag="b2h")
            nc.sync.dma_start(b2h[:], b2_ext.ap().rearrange("(h p) -> p h", p=64))
            bp = constp.tile([128, CT], F32, tag="bp")
            nc.sync.dma_start(bp[:], bp_ext.ap().rearrange("(c p) -> p c", p=128))

            qT = [qTp.tile([128, R], F16, tag=f"qT{p}", name=f"qT{p}") for p in range(H // 2)]
            o = [op_.tile([64, R], F16, tag=f"o{h}", name=f"o{h}") for h in range(H)]

            # ---------------- Phase 1: LN1 + K/V/Q projections ----------
            with (
                tc.tile_pool(name="xn", bufs=1) as xnp,
                tc.tile_pool(name="w1", bufs=1) as w1p,
                tc.tile_pool(name="tmp1", bufs=3) as tmp1p,
                tc.tile_pool(name="small1", bufs=1) as small1p,
                tc.tile_pool(name="ev1", bufs=4) as ev1p,
                tc.tile_pool(name="xt", bufs=1) as xtp,
                tc.tile_pool(name="ps_stat", bufs=1, space="PSUM") as ps_stat,
                tc.tile_pool(name="ps_bc", bufs=1, space="PSUM") as ps_bc,
                tc.tile_pool(name="ps_mm", bufs=3, space="PSUM") as ps_mm,
            ):
                xt = []
                for c in range(CT):
                    t = xtp.tile([128, R], F16, tag=f"xt{c}")
                    nc.sync.dma_start(t[:], xT_ext[128 * c : 128 * (c + 1), :])
                    xt.append(t)

                # resident fp16 weights
                wqk_sb = []
                wv_sb = []
                for c in range(CT):
                    t = w1p.tile([128, 2 * C], F16, tag=f"wqk{c}", name=f"wqk{c}")
                    nc.sync.dma_start(t[:], wqk_ext[128 * c : 128 * (c + 1), :])
                    wqk_sb.append(t)
                    t = w1p.tile([128, C], F16, tag=f"wv{c}", name=f"wv{c}")
                    nc.sync.dma_start(t[:], wv_ext[128 * c : 128 * (c + 1), :])
                    wv_sb.append(t)

                # LN1 stats: sums of x and x^2 over C (partition axis)
                sx_ps = ps_stat.tile([1, R], F32, tag="sx")
                sq_ps = ps_stat.tile([1, R], F32, tag="sq")
                for c in range(CT):
                    nc.tensor.matmul(
                        sx_ps[:], ones_col[:], xt[c][:], start=(c == 0), stop=(c == CT - 1)
                    )
                for c in range(CT):
                    xsq = tmp1p.tile([128, R], F16, tag="xsq")
                    nc.scalar.activation(xsq[:], xt[c][:], AF.Square)
                    nc.tensor.matmul(
                        sq_ps[:], ones_col[:], xsq[:], start=(c == 0), stop=(c == CT - 1)
                    )

                mu = small1p.tile([1, R], F16, tag="mu")
                nc.vector.tensor_scalar_mul(mu[:], sx_ps[:], 1.0 / C)
                m2 = small1p.tile([1, R], F32, tag="m2")
                nc.vector.tensor_scalar_mul(m2[:], sq_ps[:], 1.0 / C)
                musq = small1p.tile([1, R], F32, tag="musq")
                nc.vector.tensor_tensor(musq[:], mu[:], mu[:], op=ALU.mult)
                var = small1p.tile([1, R], F32, tag="var")
                nc.vector.tensor_tensor(var[:], m2[:], musq[:], op=ALU.subtract)
                lv = small1p.tile([1, R], F32, tag="lv")
                nc.scalar.activation(lv[:], var[:], AF.Ln, bias=eps_t[:])
                rsig = small1p.tile([1, R], F16, tag="rsig")
                nc.scalar.activation(rsig[:], lv[:], AF.Exp, scale=-0.5)

                bmu_ps = ps_bc.tile([128, R], F32, tag="bmu")
                nc.tensor.matmul(bmu_ps[:], ones_row[:], mu[:], start=True, stop=True)
                brs_ps = ps_bc.tile([128, R], F32, tag="brs")
                nc.tensor.matmul(brs_ps[:], ones_row[:], rsig[:], start=True, stop=True)

                xn = []
                for c in range(CT):
                    t1 = tmp1p.tile([128, R], F32, tag="lt1")
                    nc.vector.tensor_tensor(t1[:], xt[c][:], bmu_ps[:], op=ALU.subtract)
                    t2 = tmp1p.tile([128, R], F32, tag="lt2")
                    nc.vector.tensor_tensor(t2[:], t1[:], brs_ps[:], op=ALU.mult)
                    t3 = xnp.tile([128, R], F16, tag=f"xn{c}")
                    nc.vector.tensor_scalar(
                        t3[:], t2[:], g1[:, c : c + 1], b1[:, c : c + 1],
                        op0=ALU.mult, op1=ALU.add,
                    )
                    xn.append(t3)

                # k^T = w_k^T @ xn^T -> [C, R] feature-major, then gather
                for kf in range(CT):
                    kps = ps_mm.tile([128, R], F32, tag="kvps")
                    for c in range(CT):
                        nc.tensor.matmul(
                            kps[:],
                            wqk_sb[c][:, C + 128 * kf : C + 128 * (kf + 1)],
                            xn[c][:],
                            start=(c == 0),
                            stop=(c == CT - 1),
                        )
                    ksb = ev1p.tile([128, R], F16, tag="kev")
                    nc.scalar.copy(ksb[:], kps[:])
                    nc.sync.dma_start(k_in[128 * kf : 128 * (kf + 1), :], ksb[:])

                nc.gpsimd.collective_compute(
                    "AllGather",
                    ALU.bypass,
                    ins=[k_in[:]],
                    outs=[k_out[:]],
                    replica_groups=[[0, 1, 2, 3], [4, 5, 6, 7]],
                )

                # v = xn @ w_v -> [R, H, 65] row-major with ones cols, gather
                for rt in range(R // 128):
                    for vf in range(2):
                        vps = ps_mm.tile([128, R], F32, tag="kvps")
                        for c in range(CT):
                            nc.tensor.matmul(
                                vps[:],
                                xn[c][:, 128 * rt : 128 * (rt + 1)],
                                wv_sb[c][:, 512 * vf : 512 * (vf + 1)],
                                start=(c == 0),
                                stop=(c == CT - 1),
                            )
                        vsb = ev1p.tile([128, 8, HD + 1], F16, tag="vev")
                        nc.scalar.copy(
                            vsb[:, :, 0:HD],
                            vps[:].rearrange("p (h d) -> p h d", d=HD),
                        )
                        nc.vector.memset(vsb[:, :, HD : HD + 1], 1.0)
                        nc.sync.dma_start(
                            v_in[128 * rt : 128 * (rt + 1), 8 * vf : 8 * (vf + 1), :],
                            vsb[:],
                        )

                nc.gpsimd.collective_compute(
                    "AllGather",
                    ALU.bypass,
                    ins=[v_in[:]],
                    outs=[v_out[:]],
                    replica_groups=[[0, 1, 2, 3], [4, 5, 6, 7]],
                )

                # q^T = w_q^T @ xn^T -> [C, R] (overlaps the all-gathers)
                for qf in range(CT):
                    qps = ps_mm.tile([128, R], F32, tag="kvps")
                    for c in range(CT):
                        nc.tensor.matmul(
                            qps[:],
                            wqk_sb[c][:, 128 * qf : 128 * (qf + 1)],
                            xn[c][:],
                            start=(c == 0),
                            stop=(c == CT - 1),
                        )
                    nc.scalar.copy(qT[qf][:], qps[:])

            # ---------------- Phase 2: attention -------------------------
            # All score/exp work first (depends only on the K gather), then
            # all AV work (needs the V gather) — keeps the PE busy through
            # the V all-gather window.  att slots throttle the lookahead.
            with (
                tc.tile_pool(name="kpair", bufs=2) as kpp,
                tc.tile_pool(name="vpair", bufs=3) as vpp,
                tc.tile_pool(name="att", bufs=48) as attp,
                tc.tile_pool(name="rec", bufs=4) as recp,
                tc.tile_pool(name="ps_sc", bufs=2, space="PSUM") as ps_sc,
                tc.tile_pool(name="ps_av", bufs=2, space="PSUM") as ps_av,
                tc.tile_pool(name="ps_nb", bufs=2, space="PSUM") as ps_nb,
            ):
                NP = H // 2
                att = [None] * NP

                for p in range(NP):
                    kpair = kpp.tile([128, GROUP * R], F16, tag="kpair", name=f"kp{p}")
                    nc.sync.dma_start(
                        kpair[:].rearrange("p (r n) -> p r n", r=GROUP),
                        k_out[:, 128 * p : 128 * (p + 1), :].rearrange("r p n -> p r n"),
                    )
                    att[p] = [[], []]
                    for g in range(NKC // 2):
                        scp = [None, None]
                        for hi in range(2):
                            base = 64 * hi
                            scp[hi] = ps_sc.tile(
                                [128, 1024], F32, tag="scps", name=f"sc{p}_{g}_{hi}"
                            )
                            for cc in range(2):
                                j = 2 * g + cc
                                nc.tensor.matmul(
                                    scp[hi][:, 512 * cc : 512 * (cc + 1)],
                                    kpair[base : base + 64, 128 * j : 128 * (j + 1)],
                                    qT[p][base : base + 64, :],
                                    start=True,
                                    stop=True,
                                )
                        for hi in range(2):
                            at = attp.tile([128, 1024], F16, tag="att")
                            nc.scalar.activation(at[:], scp[hi][:], AF.Exp, scale=SCALE)
                            att[p][hi].append(at)

                for p in range(NP):
                    vpair = vpp.tile(
                        [128, NKC, 2 * (HD + 1)], F16, tag="vpair", name=f"vp{p}"
                    )
                    nc.sync.dma_start(
                        vpair[:],
                        v_out[:]
                        .rearrange("r (j p) h d -> p (r j) (h d)", p=128)[
                            :, :, (HD + 1) * 2 * p : (HD + 1) * 2 * (p + 1)
                        ],
                    )
                    for hi in range(2):
                        h = 2 * p + hi
                        avps = ps_av.tile([HD + 1, R], F32, tag="avps", name=f"av{h}")
                        for j in range(NKC):
                            nc.tensor.matmul(
                                avps[:],
                                vpair[:, j, (HD + 1) * hi : (HD + 1) * (hi + 1)],
                                att[p][hi][j // 2][:, 512 * (j % 2) : 512 * (j % 2 + 1)],
                                start=(j == 0),
                                stop=(j == NKC - 1),
                            )
                        den = recp.tile([1, R], F32, tag="den")
                        nc.vector.tensor_copy(den[:], avps[HD : HD + 1, :])
                        recf = recp.tile([1, R], F32, tag="recf")
                        with nc.allow_low_precision(reason="softmax denom bcast"):
                            nc.vector.reciprocal_approx_fast(recf[:], den[:])
                        rec = recp.tile([1, R], F16, tag="rec")
                        nc.vector.tensor_copy(rec[:], recf[:])
                        nbps = ps_nb.tile([64, R], F32, tag="nbps", name=f"nb{h}")
                        nc.tensor.matmul(
                            nbps[:], ones_row[:, 0:64], rec[:], start=True, stop=True
                        )
                        nc.vector.tensor_copy(o[h][:], avps[0:HD, :])
                        nc.vector.tensor_tensor(o[h][:], o[h][:], nbps[:], op=ALU.mult)
                    att[p] = None

            # ---------------- Phase 3: LN2 + projection ------------------
            with (
                tc.tile_pool(name="wp", bufs=1) as wpp,
                tc.tile_pool(name="tmp2", bufs=4) as tmp2p,
                tc.tile_pool(name="small2", bufs=1) as small2p,
                tc.tile_pool(name="ln2o", bufs=1) as ln2op,
                tc.tile_pool(name="yev", bufs=3) as yevp,
                tc.tile_pool(name="ps_stat2", bufs=1, space="PSUM") as ps_stat2,
                tc.tile_pool(name="ps_bc2", bufs=1, space="PSUM") as ps_bc2,
                tc.tile_pool(name="ps_y", bufs=3, space="PSUM") as ps_y,
            ):
                wp_sb = []
                for h in range(H):
                    t = wpp.tile([64, C], F16, tag=f"wp{h}", name=f"wp{h}")
                    nc.sync.dma_start(t[:], wp_ext[HD * h : HD * (h + 1), :])
                    wp_sb.append(t)

                s2_ps = ps_stat2.tile([1, R], F32, tag="s2")
                q2_ps = ps_stat2.tile([1, R], F32, tag="q2")
                for h in range(H):
                    nc.tensor.matmul(
                        s2_ps[:], ones_col[0:64, :], o[h][:],
                        start=(h == 0), stop=(h == H - 1),
                    )
                for h in range(H):
                    osq = tmp2p.tile([64, R], F16, tag="osq")
                    nc.scalar.activation(osq[:], o[h][:], AF.Square)
                    nc.tensor.matmul(
                        q2_ps[:], ones_col[0:64, :], osq[:],
                        start=(h == 0), stop=(h == H - 1),
                    )

                mu2 = small2p.tile([1, R], F16, tag="mu2")
                nc.vector.tensor_scalar_mul(mu2[:], s2_ps[:], 1.0 / C)
                m22 = small2p.tile([1, R], F32, tag="m22")
                nc.vector.tensor_scalar_mul(m22[:], q2_ps[:], 1.0 / C)
                musq2 = small2p.tile([1, R], F32, tag="musq2")
                nc.vector.tensor_tensor(musq2[:], mu2[:], mu2[:], op=ALU.mult)
                var2 = small2p.tile([1, R], F32, tag="var2")
                nc.vector.tensor_tensor(var2[:], m22[:], musq2[:], op=ALU.subtract)
                lv2 = small2p.tile([1, R], F32, tag="lv2")
                nc.scalar.activation(lv2[:], var2[:], AF.Ln, bias=eps_t[:])
                rsig2 = small2p.tile([1, R], F16, tag="rsig2")
                nc.scalar.activation(rsig2[:], lv2[:], AF.Exp, scale=-0.5)

                bmu2_ps = ps_bc2.tile([64, R], F32, tag="bmu2")
                nc.tensor.matmul(
                    bmu2_ps[:], ones_row[:, 0:64], mu2[:], start=True, stop=True
                )
                brs2_ps = ps_bc2.tile([64, R], F32, tag="brs2")
                nc.tensor.matmul(
                    brs2_ps[:], ones_row[:, 0:64], rsig2[:], start=True, stop=True
                )

                ln2o = []
                for h in range(H):
                    t1 = tmp2p.tile([64, R], F32, tag="l2t1")
                    nc.vector.tensor_tensor(t1[:], o[h][:], bmu2_ps[:], op=ALU.subtract)
                    t2 = tmp2p.tile([64, R], F32, tag="l2t2")
                    nc.vector.tensor_tensor(t2[:], t1[:], brs2_ps[:], op=ALU.mult)
                    t3 = ln2op.tile([64, R], F16, tag=f"ln2o{h}", name=f"ln2o{h}")
                    nc.vector.tensor_scalar(
                        t3[:], t2[:], g2h[:, h : h + 1], b2h[:, h : h + 1],
                        op0=ALU.mult, op1=ALU.add,
                    )
                    ln2o.append(t3)

                for of in range(CT):
                    yps = ps_y.tile([128, R], F32, tag="yps")
                    for h in range(H):
                        nc.tensor.matmul(
                            yps[:],
                            wp_sb[h][:, 128 * of : 128 * (of + 1)],
                            ln2o[h][:],
                            start=(h == 0),
                            stop=(h == H - 1),
                        )
                    ysb = yevp.tile([128, R], F32, tag="yev")
                    nc.scalar.activation(
                        ysb[:], yps[:], AF.Identity, bias=bp[:, of : of + 1]
                    )
                    nc.sync.dma_start(out_ext[128 * of : 128 * (of + 1), :], ysb[:])

    nc.compile()
    return nc


def kernel(x, ln1_g, ln1_b, w_qk, w_v, ln2_g, ln2_b, w_proj, b_proj):
    global _cached_nc, last_exec_time_ns
    if _cached_nc is None:
        _cached_nc = _build()
    nc = _cached_nc

    xr = np.asarray(x, np.float32).reshape(B * N, C).astype(np.float16)
    shared = {
        "w_qk": np.asarray(w_qk, np.float32).astype(np.float16),
        "w_v": np.asarray(w_v, np.float32).astype(np.float16),
        "w_proj": np.asarray(w_proj, np.float32).astype(np.float16),
        "ln1_g": np.ascontiguousarray(ln1_g, np.float32),
        "ln1_b": np.ascontiguousarray(ln1_b, np.float32),
        "ln2_g": np.ascontiguousarray(ln2_g, np.float32),
        "ln2_b": np.ascontiguousarray(ln2_b, np.float32),
        "b_proj": np.ascontiguousarray(b_proj, np.float32),
    }
    in_maps = []
    for i in range(NCORES):
        xT_i = np.ascontiguousarray(xr[R * i : R * (i + 1), :].T)
        in_maps.append({"xT": xT_i, **shared})

    res = run_bass_kernel_spmd(nc, in_maps, core_ids=list(range(NCORES)))
    last_exec_time_ns = res.exec_time_ns

    y = np.empty((B * N, C), np.float32)
    for i in range(NCORES):
        y[R * i : R * (i + 1), :] = res.results[i]["out"].T
    return y.reshape(B, N, C)


# revision 12
# speedup vs baseline: 1.7551x; 1.0082x over previous
"""Distributed Trainium2 kernel for the dense-transformer attention block:

    xn = LN(x); q,k = xn @ w_qk; v = xn @ w_v
    out = softmax(q k^T / sqrt(hd)) v ; out = LN(out) @ w_proj + b_proj

Sharding: the (B=2, N=2048) token axis is flattened to 4096 rows, 512 per
core (cores 0-3 own batch 0, cores 4-7 batch 1).  Each core runs LN1 and
the K/V projections on its rows, all-gathers K^T and V (fp16) inside its
4-core batch group, computes all 16 heads of attention for its own 512
query rows, then LN2 and the output projection — every FLOP except the
gathers is done exactly once across the chip.

On-chip data lives in a transposed ("feature on partition") layout: the
host feeds x^T per core and transposes the returned y^T back (free).
Matmul operands are fp16 (full PE rate + fast weight load); PSUM
accumulation and LayerNorm statistics stay fp32.  Softmax denominators
come free from a ones-column appended to V (row 64 of the AV output);
no max-subtraction is needed because scores are ~N(0,1).
"""

import numpy as np

import concourse.bass as bass
import concourse.mybir as mybir
import concourse.tile as tile
from concourse import bacc
from concourse.bass_utils import run_bass_kernel_spmd

B, N, C = 2, 2048, 1024
H, HD = 16, 64
NCORES = 8
R = (B * N) // NCORES  # 512 rows per core
GROUP = NCORES // B  # 4 cores per batch
NKC = (GROUP * R) // 128  # 16 k-chunks of 128
EPS = 1e-5
SCALE = HD**-0.5

F32 = mybir.dt.float32
F16 = mybir.dt.float16
AF = mybir.ActivationFunctionType
ALU = mybir.AluOpType

CT = C // 128  # 8 C tiles

last_exec_time_ns = None
_cached_nc = None


def _build():
    nc = bacc.Bacc("TRN2", target_bir_lowering=False, debug=False, num_devices=NCORES)

    xT_ext = nc.dram_tensor("xT", [C, R], F16, kind="ExternalInput")
    wqk_ext = nc.dram_tensor("w_qk", [C, 2 * C], F16, kind="ExternalInput")
    wv_ext = nc.dram_tensor("w_v", [C, C], F16, kind="ExternalInput")
    wp_ext = nc.dram_tensor("w_proj", [C, C], F16, kind="ExternalInput")
    g1_ext = nc.dram_tensor("ln1_g", [C], F32, kind="ExternalInput")
    b1_ext = nc.dram_tensor("ln1_b", [C], F32, kind="ExternalInput")
    g2_ext = nc.dram_tensor("ln2_g", [C], F32, kind="ExternalInput")
    b2_ext = nc.dram_tensor("ln2_b", [C], F32, kind="ExternalInput")
    bp_ext = nc.dram_tensor("b_proj", [C], F32, kind="ExternalInput")
    out_ext = nc.dram_tensor("out", [C, R], F32, kind="ExternalOutput")

    # k^T split in two halves so the first gather starts earlier
    k_in = [
        nc.dram_tensor(f"k_in{i}", [C // 2, R], F16) for i in range(2)
    ]  # k^T, feature-major
    k_out = [nc.dram_tensor(f"k_out{i}", [GROUP, C // 2, R], F16) for i in range(2)]
    v_in = nc.dram_tensor("v_in", [R, H, HD + 1], F16)  # v + ones col per head
    v_out = nc.dram_tensor("v_out", [GROUP, R, H, HD + 1], F16)

    with tile.TileContext(nc) as tc:
        with (
            tc.tile_pool(name="const", bufs=1) as constp,
            tc.tile_pool(name="qT", bufs=1) as qTp,
            tc.tile_pool(name="o", bufs=1) as op_,
            tc.tile_pool(name="ps_st2", bufs=1, space="PSUM") as ps_st2,
        ):
            s2_ps = ps_st2.tile([1, R], F32, tag="s2")
            q2_ps = ps_st2.tile([1, R], F32, tag="q2")
            ones_col = constp.tile([128, 1], F16, tag="onesc")
            nc.vector.memset(ones_col[:], 1.0)
            ones_row = constp.tile([1, 128], F16, tag="onesr")
            nc.vector.memset(ones_row[:], 1.0)
            eps_t = constp.tile([1, 1], F32, tag="epsc")
            nc.sync.dma_start(
                eps_t[:], nc.inline_tensor(np.full((1, 1), EPS, np.float32), "epsc").ap()
            )
            g1 = constp.tile([128, CT], F32, tag="g1")
            nc.sync.dma_start(g1[:], g1_ext.ap().rearrange("(c p) -> p c", p=128))
            b1 = constp.tile([128, CT], F32, tag="b1")
            nc.sync.dma_start(b1[:], b1_ext.ap().rearrange("(c p) -> p c", p=128))
            g2h = constp.tile([64, H], F32, tag="g2h")
            nc.sync.dma_start(g2h[:], g2_ext.ap().rearrange("(h p) -> p h", p=64))
            b2h = constp.tile([64, H], F32, tag="b2h")
            nc.sync.dma_start(b2h[:], b2_ext.ap().rearrange("(h p) -> p h", p=64))
            bp = constp.tile([128, CT], F32, tag="bp")
            nc.sync.dma_start(bp[:], bp_ext.ap().rearrange("(c p) -> p c", p=128))

            qT = [qTp.tile([128, R], F16, tag=f"qT{p}", name=f"qT{p}") for p in range(H // 2)]
            o = [op_.tile([64, R], F16, tag=f"o{h}", name=f"o{h}") for h in range(H)]

            # ---------------- Phase 1: LN1 + K/V/Q projections ----------
            with (
                tc.tile_pool(name="xn", bufs=1) as xnp,
                tc.tile_pool(name="w1", bufs=1) as w1p,
                tc.tile_pool(name="tmp1", bufs=3) as tmp1p,
                tc.tile_pool(name="small1", bufs=1) as small1p,
                tc.tile_pool(name="ev1", bufs=4) as ev1p,
                tc.tile_pool(name="xt", bufs=1) as xtp,
                tc.tile_pool(name="ps_stat", bufs=1, space="PSUM") as ps_stat,
                tc.tile_pool(name="ps_bc", bufs=1, space="PSUM") as ps_bc,
                tc.tile_pool(name="ps_mm", bufs=2, space="PSUM") as ps_mm,
            ):
                xt = []
                for c in range(CT):
                    t = xtp.tile([128, R], F16, tag=f"xt{c}")
                    nc.sync.dma_start(t[:], xT_ext[128 * c : 128 * (c + 1), :])
                    xt.append(t)

                # resident fp16 weights
                wqk_sb = []
                wv_sb = []
                for c in range(CT):
                    t = w1p.tile([128, 2 * C], F16, tag=f"wqk{c}", name=f"wqk{c}")
                    nc.sync.dma_start(t[:], wqk_ext[128 * c : 128 * (c + 1), :])
                    wqk_sb.append(t)
                    t = w1p.tile([128, C], F16, tag=f"wv{c}", name=f"wv{c}")
                    nc.sync.dma_start(t[:], wv_ext[128 * c : 128 * (c + 1), :])
                    wv_sb.append(t)

                # LN1 stats: sums of x and x^2 over C (partition axis)
                sx_ps = ps_stat.tile([1, R], F32, tag="sx")
                sq_ps = ps_stat.tile([1, R], F32, tag="sq")
                for c in range(CT):
                    nc.tensor.matmul(
                        sx_ps[:], ones_col[:], xt[c][:], start=(c == 0), stop=(c == CT - 1)
                    )
                for c in range(CT):
                    xsq = tmp1p.tile([128, R], F16, tag="xsq")
                    nc.scalar.activation(xsq[:], xt[c][:], AF.Square)
                    nc.tensor.matmul(
                        sq_ps[:], ones_col[:], xsq[:], start=(c == 0), stop=(c == CT - 1)
                    )

                mu = small1p.tile([1, R], F16, tag="mu")
                nc.vector.tensor_scalar_mul(mu[:], sx_ps[:], 1.0 / C)
                m2 = small1p.tile([1, R], F32, tag="m2")
                nc.vector.tensor_scalar_mul(m2[:], sq_ps[:], 1.0 / C)
                musq = small1p.tile([1, R], F32, tag="musq")
                nc.vector.tensor_tensor(musq[:], mu[:], mu[:], op=ALU.mult)
                var = small1p.tile([1, R], F32, tag="var")
                nc.vector.tensor_tensor(var[:], m2[:], musq[:], op=ALU.subtract)
                lv = small1p.tile([1, R], F32, tag="lv")
                nc.scalar.activation(lv[:], var[:], AF.Ln, bias=eps_t[:])
                rsig = small1p.tile([1, R], F16, tag="rsig")
                nc.scalar.activation(rsig[:], lv[:], AF.Exp, scale=-0.5)

                bmu_ps = ps_bc.tile([128, R], F32, tag="bmu")
                nc.tensor.matmul(bmu_ps[:], ones_row[:], mu[:], start=True, stop=True)
                brs_ps = ps_bc.tile([128, R], F32, tag="brs")
                nc.tensor.matmul(brs_ps[:], ones_row[:], rsig[:], start=True, stop=True)

                xn = []
                for c in range(CT):
                    t1 = tmp1p.tile([128, R], F32, tag="lt1")
                    nc.vector.tensor_tensor(t1[:], xt[c][:], bmu_ps[:], op=ALU.subtract)
                    t2 = tmp1p.tile([128, R], F32, tag="lt2")
                    nc.vector.tensor_tensor(t2[:], t1[:], brs_ps[:], op=ALU.mult)
                    t3 = xnp.tile([128, R], F16, tag=f"xn{c}")
                    nc.vector.tensor_scalar(
                        t3[:], t2[:], g1[:, c : c + 1], b1[:, c : c + 1],
                        op0=ALU.mult, op1=ALU.add,
                    )
                    xn.append(t3)

                # k^T = w_k^T @ xn^T -> [C, R] feature-major, gathered in
                # two halves so comms start as early as possible
                for half in range(2):
                    for kfh in range(CT // 2):
                        kf = half * (CT // 2) + kfh
                        kps = ps_mm.tile([128, R], F32, tag="kvps")
                        for c in range(CT):
                            nc.tensor.matmul(
                                kps[:],
                                wqk_sb[c][:, C + 128 * kf : C + 128 * (kf + 1)],
                                xn[c][:],
                                start=(c == 0),
                                stop=(c == CT - 1),
                            )
                        ksb = ev1p.tile([128, R], F16, tag="kev")
                        nc.scalar.copy(ksb[:], kps[:])
                        nc.sync.dma_start(
                            k_in[half][128 * kfh : 128 * (kfh + 1), :], ksb[:]
                        )
                    nc.gpsimd.collective_compute(
                        "AllGather",
                        ALU.bypass,
                        ins=[k_in[half][:]],
                        outs=[k_out[half][:]],
                        replica_groups=[[0, 1, 2, 3], [4, 5, 6, 7]],
                    )

                # v = xn @ w_v -> [R, H, 65] row-major with ones cols, gather
                for rt in range(R // 128):
                    for vf in range(2):
                        vps = ps_mm.tile([128, R], F32, tag="kvps")
                        for c in range(CT):
                            nc.tensor.matmul(
                                vps[:],
                                xn[c][:, 128 * rt : 128 * (rt + 1)],
                                wv_sb[c][:, 512 * vf : 512 * (vf + 1)],
                                start=(c == 0),
                                stop=(c == CT - 1),
                            )
                        vsb = ev1p.tile([128, 8, HD + 1], F16, tag="vev")
                        nc.scalar.copy(
                            vsb[:, :, 0:HD],
                            vps[:].rearrange("p (h d) -> p h d", d=HD),
                        )
                        nc.vector.memset(vsb[:, :, HD : HD + 1], 1.0)
                        nc.sync.dma_start(
                            v_in[128 * rt : 128 * (rt + 1), 8 * vf : 8 * (vf + 1), :],
                            vsb[:],
                        )

                nc.gpsimd.collective_compute(
                    "AllGather",
                    ALU.bypass,
                    ins=[v_in[:]],
                    outs=[v_out[:]],
                    replica_groups=[[0, 1, 2, 3], [4, 5, 6, 7]],
                )

                # q^T = w_q^T @ xn^T -> [C, R] (overlaps the all-gathers)
                for qf in range(CT):
                    qps = ps_mm.tile([128, R], F32, tag="kvps")
                    for c in range(CT):
                        nc.tensor.matmul(
                            qps[:],
                            wqk_sb[c][:, 128 * qf : 128 * (qf + 1)],
                            xn[c][:],
                            start=(c == 0),
                            stop=(c == CT - 1),
                        )
                    nc.scalar.copy(qT[qf][:], qps[:])

            # ---------------- Phase 2: attention -------------------------
            # All score/exp work first (depends only on the K gather), then
            # all AV work (needs the V gather) — keeps the PE busy through
            # the V all-gather window.  att slots throttle the lookahead.
            with (
                tc.tile_pool(name="kpair", bufs=2) as kpp,
                tc.tile_pool(name="vpair", bufs=3) as vpp,
                tc.tile_pool(name="att", bufs=48) as attp,
                tc.tile_pool(name="rec", bufs=4) as recp,
                tc.tile_pool(name="ps_sc", bufs=2, space="PSUM") as ps_sc,
                tc.tile_pool(name="ps_av", bufs=2, space="PSUM") as ps_av,
            ):
                NP = H // 2
                att = [None] * NP

                for p in range(NP):
                    kpair = kpp.tile([128, GROUP * R], F16, tag="kpair", name=f"kp{p}")
                    half, ph = divmod(p, CT // 2)
                    nc.sync.dma_start(
                        kpair[:].rearrange("p (r n) -> p r n", r=GROUP),
                        k_out[half][:, 128 * ph : 128 * (ph + 1), :].rearrange(
                            "r p n -> p r n"
                        ),
                    )
                    att[p] = [[], []]
                    for g in range(NKC // 2):
                        scp = [None, None]
                        for hi in range(2):
                            base = 64 * hi
                            scp[hi] = ps_sc.tile(
                                [128, 1024], F32, tag="scps", name=f"sc{p}_{g}_{hi}"
                            )
                            for cc in range(2):
                                j = 2 * g + cc
                                nc.tensor.matmul(
                                    scp[hi][:, 512 * cc : 512 * (cc + 1)],
                                    kpair[base : base + 64, 128 * j : 128 * (j + 1)],
                                    qT[p][base : base + 64, :],
                                    start=True,
                                    stop=True,
                                )
                        for hi in range(2):
                            at = attp.tile([128, 1024], F16, tag="att")
                            nc.scalar.activation(at[:], scp[hi][:], AF.Exp, scale=SCALE)
                            att[p][hi].append(at)

                for p in range(NP):
                    vpair = vpp.tile(
                        [128, NKC, 2 * (HD + 1)], F16, tag="vpair", name=f"vp{p}"
                    )
                    nc.sync.dma_start(
                        vpair[:],
                        v_out[:]
                        .rearrange("r (j p) h d -> p (r j) (h d)", p=128)[
                            :, :, (HD + 1) * 2 * p : (HD + 1) * 2 * (p + 1)
                        ],
                    )
                    for hi in range(2):
                        h = 2 * p + hi
                        avps = ps_av.tile([HD + 1, R], F32, tag="avps", name=f"av{h}")
                        for j in range(NKC):
                            nc.tensor.matmul(
                                avps[:],
                                vpair[:, j, (HD + 1) * hi : (HD + 1) * (hi + 1)],
                                att[p][hi][j // 2][:, 512 * (j % 2) : 512 * (j % 2 + 1)],
                                start=(j == 0),
                                stop=(j == NKC - 1),
                            )
                        den = recp.tile([1, R], F32, tag="den")
                        nc.vector.tensor_copy(den[:], avps[HD : HD + 1, :])
                        recf = recp.tile([1, R], F32, tag="recf")
                        with nc.allow_low_precision(reason="softmax denom bcast"):
                            nc.vector.reciprocal_approx_fast(recf[:], den[:])
                        rec = recp.tile([1, R], F16, tag="rec")
                        nc.vector.tensor_copy(rec[:], recf[:])
                        nc.vector.tensor_copy(o[h][:], avps[0:HD, :])
                        # broadcast 1/denom into the (now free) avps bank
                        nc.tensor.matmul(
                            avps[0:HD, :], ones_row[:, 0:HD], rec[:],
                            start=True, stop=True,
                        )
                        nc.vector.tensor_tensor(o[h][:], o[h][:], avps[0:HD, :], op=ALU.mult)
                        # LN2 statistics accumulate while attention proceeds
                        nc.tensor.matmul(
                            s2_ps[:], ones_col[0:64, :], o[h][:],
                            start=(h == 0), stop=(h == H - 1),
                        )
                        osq = recp.tile([64, R], F16, tag="osq")
                        nc.vector.tensor_tensor(osq[:], o[h][:], o[h][:], op=ALU.mult)
                        nc.tensor.matmul(
                            q2_ps[:], ones_col[0:64, :], osq[:],
                            start=(h == 0), stop=(h == H - 1),
                        )
                    att[p] = None

            # ---------------- Phase 3: LN2 + projection ------------------
            with (
                tc.tile_pool(name="wp", bufs=1) as wpp,
                tc.tile_pool(name="tmp2", bufs=4) as tmp2p,
                tc.tile_pool(name="small2", bufs=1) as small2p,
                tc.tile_pool(name="ln2o", bufs=1) as ln2op,
                tc.tile_pool(name="yev", bufs=3) as yevp,
                tc.tile_pool(name="ps_bc2", bufs=1, space="PSUM") as ps_bc2,
                tc.tile_pool(name="ps_y", bufs=4, space="PSUM") as ps_y,
            ):
                wp_sb = []
                for h in range(H):
                    t = wpp.tile([64, C], F16, tag=f"wp{h}", name=f"wp{h}")
                    nc.sync.dma_start(t[:], wp_ext[HD * h : HD * (h + 1), :])
                    wp_sb.append(t)

                mu2 = small2p.tile([1, R], F16, tag="mu2")
                nc.vector.tensor_scalar_mul(mu2[:], s2_ps[:], 1.0 / C)
                m22 = small2p.tile([1, R], F32, tag="m22")
                nc.vector.tensor_scalar_mul(m22[:], q2_ps[:], 1.0 / C)
                musq2 = small2p.tile([1, R], F32, tag="musq2")
                nc.vector.tensor_tensor(musq2[:], mu2[:], mu2[:], op=ALU.mult)
                var2 = small2p.tile([1, R], F32, tag="var2")
                nc.vector.tensor_tensor(var2[:], m22[:], musq2[:], op=ALU.subtract)
                lv2 = small2p.tile([1, R], F32, tag="lv2")
                nc.scalar.activation(lv2[:], var2[:], AF.Ln, bias=eps_t[:])
                rsig2 = small2p.tile([1, R], F16, tag="rsig2")
                nc.scalar.activation(rsig2[:], lv2[:], AF.Exp, scale=-0.5)

                bmu2_ps = ps_bc2.tile([64, R], F32, tag="bmu2")
                nc.tensor.matmul(
                    bmu2_ps[:], ones_row[:, 0:64], mu2[:], start=True, stop=True
                )
                brs2_ps = ps_bc2.tile([64, R], F32, tag="brs2")
                nc.tensor.matmul(
                    brs2_ps[:], ones_row[:, 0:64], rsig2[:], start=True, stop=True
                )

                ln2o = []
                for h in range(H):
                    t1 = tmp2p.tile([64, R], F32, tag="l2t1")
                    nc.vector.tensor_tensor(t1[:], o[h][:], bmu2_ps[:], op=ALU.subtract)
                    t2 = tmp2p.tile([64, R], F32, tag="l2t2")
                    nc.vector.tensor_tensor(t2[:], t1[:], brs2_ps[:], op=ALU.mult)
                    t3 = ln2op.tile([64, R], F16, tag=f"ln2o{h}", name=f"ln2o{h}")
                    nc.vector.tensor_scalar(
                        t3[:], t2[:], g2h[:, h : h + 1], b2h[:, h : h + 1],
                        op0=ALU.mult, op1=ALU.add,
                    )
                    ln2o.append(t3)

                # h-outer projection: matmuls for head h start as soon as
                # ln2o[h] exists, overlapping the LN2 apply chain
                for half in range(2):
                    yps_t = [
                        ps_y.tile([128, R], F32, tag="yps", name=f"yps{half}_{i}")
                        for i in range(4)
                    ]
                    for h in range(H):
                        for i in range(4):
                            of = 4 * half + i
                            nc.tensor.matmul(
                                yps_t[i][:],
                                wp_sb[h][:, 128 * of : 128 * (of + 1)],
                                ln2o[h][:],
                                start=(h == 0),
                                stop=(h == H - 1),
                            )
                    for i in range(4):
                        of = 4 * half + i
                        ysb = yevp.tile([128, R], F32, tag="yev")
                        nc.scalar.activation(
                            ysb[:], yps_t[i][:], AF.Identity, bias=bp[:, of : of + 1]
                        )
                        nc.sync.dma_start(
                            out_ext[128 * of : 128 * (of + 1), :], ysb[:]
                        )

    nc.compile()
    return nc


def kernel(x, ln1_g, ln1_b, w_qk, w_v, ln2_g, ln2_b, w_proj, b_proj):
    global _cached_nc, last_exec_time_ns
    if _cached_nc is None:
        _cached_nc = _build()
    nc = _cached_nc

    xr = np.asarray(x, np.float32).reshape(B * N, C).astype(np.float16)
    shared = {
        "w_qk": np.asarray(w_qk, np.float32).astype(np.float16),
        "w_v": np.asarray(w_v, np.float32).astype(np.float16),
        "w_proj": np.asarray(w_proj, np.float32).astype(np.float16),
        "ln1_g": np.ascontiguousarray(ln1_g, np.float32),
        "ln1_b": np.ascontiguousarray(ln1_b, np.float32),
        "ln2_g": np.ascontiguousarray(ln2_g, np.float32),
        "ln2_b": np.ascontiguousarray(ln2_b, np.float32),
        "b_proj": np.ascontiguousarray(b_proj, np.float32),
    }
    in_maps = []
    for i in range(NCORES):
        xT_i = np.ascontiguousarray(xr[R * i : R * (i + 1), :].T)
        in_maps.append({"xT": xT_i, **shared})

    res = run_bass_kernel_spmd(nc, in_maps, core_ids=list(range(NCORES)))
    last_exec_time_ns = res.exec_time_ns

    y = np.empty((B * N, C), np.float32)
    for i in range(NCORES):
        y[R * i : R * (i + 1), :] = res.results[i]["out"].T
    return y.reshape(B, N, C)


# revision 13
# speedup vs baseline: 1.9402x; 1.1055x over previous
"""Distributed Trainium2 kernel for the dense-transformer attention block:

    xn = LN(x); q,k = xn @ w_qk; v = xn @ w_v
    out = softmax(q k^T / sqrt(hd)) v ; out = LN(out) @ w_proj + b_proj

Sharding: the (B=2, N=2048) token axis is flattened to 4096 rows, 512 per
core (cores 0-3 own batch 0, cores 4-7 batch 1).  Each core runs LN1 and
the K/V projections on its rows, all-gathers K^T and V (fp16) inside its
4-core batch group, computes all 16 heads of attention for its own 512
query rows, then LN2 and the output projection — every FLOP except the
gathers is done exactly once across the chip.

On-chip data lives in a transposed ("feature on partition") layout: the
host feeds x^T per core and transposes the returned y^T back (free).
Matmul operands are fp16 (full PE rate + fast weight load); PSUM
accumulation and LayerNorm statistics stay fp32.  Softmax denominators
come free from a ones-column appended to V (row 64 of the AV output);
no max-subtraction is needed because scores are ~N(0,1).
"""

import numpy as np

import concourse.bass as bass
import concourse.mybir as mybir
import concourse.tile as tile
from concourse import bacc
from concourse.bass_utils import run_bass_kernel_spmd

B, N, C = 2, 2048, 1024
H, HD = 16, 64
NCORES = 8
R = (B * N) // NCORES  # 512 rows per core
GROUP = NCORES // B  # 4 cores per batch
NKC = (GROUP * R) // 128  # 16 k-chunks of 128
EPS = 1e-5
SCALE = HD**-0.5

F32 = mybir.dt.float32
F16 = mybir.dt.float16
AF = mybir.ActivationFunctionType
ALU = mybir.AluOpType

CT = C // 128  # 8 C tiles

last_exec_time_ns = None
_cached_nc = None


def _build():
    nc = bacc.Bacc("TRN2", target_bir_lowering=False, debug=False, num_devices=NCORES)

    xT_ext = nc.dram_tensor("xT", [C, R], F16, kind="ExternalInput")
    wqk_ext = nc.dram_tensor("w_qk", [C, 2 * C], F16, kind="ExternalInput")
    wv_ext = nc.dram_tensor("w_v", [C, C], F16, kind="ExternalInput")
    wp_ext = nc.dram_tensor("w_proj", [C, C], F16, kind="ExternalInput")
    g1_ext = nc.dram_tensor("ln1_g", [C], F32, kind="ExternalInput")
    b1_ext = nc.dram_tensor("ln1_b", [C], F32, kind="ExternalInput")
    g2_ext = nc.dram_tensor("ln2_g", [C], F32, kind="ExternalInput")
    b2_ext = nc.dram_tensor("ln2_b", [C], F32, kind="ExternalInput")
    bp_ext = nc.dram_tensor("b_proj", [C], F32, kind="ExternalInput")
    out_ext = nc.dram_tensor("out", [C, R], F32, kind="ExternalOutput")

    # k^T split in two halves so the first gather starts earlier
    k_in = [
        nc.dram_tensor(f"k_in{i}", [C // 2, R], F16) for i in range(2)
    ]  # k^T, feature-major
    k_out = [nc.dram_tensor(f"k_out{i}", [GROUP, C // 2, R], F16) for i in range(2)]
    v_in = nc.dram_tensor("v_in", [R, H, HD + 1], F16)  # v + ones col per head
    v_out = nc.dram_tensor("v_out", [GROUP, R, H, HD + 1], F16)

    with tile.TileContext(nc) as tc:
        with (
            tc.tile_pool(name="const", bufs=1) as constp,
            tc.tile_pool(name="qT", bufs=1) as qTp,
            tc.tile_pool(name="o", bufs=1) as op_,
            tc.tile_pool(name="ps_st2", bufs=1, space="PSUM") as ps_st2,
        ):
            s2_ps = ps_st2.tile([1, R], F32, tag="s2")
            q2_ps = ps_st2.tile([1, R], F32, tag="q2")
            ones_col = constp.tile([128, 1], F16, tag="onesc")
            nc.vector.memset(ones_col[:], 1.0)
            ones_row = constp.tile([1, 128], F16, tag="onesr")
            nc.vector.memset(ones_row[:], 1.0)
            eps_t = constp.tile([1, 1], F32, tag="epsc")
            nc.sync.dma_start(
                eps_t[:], nc.inline_tensor(np.full((1, 1), EPS, np.float32), "epsc").ap()
            )
            g1 = constp.tile([128, CT], F32, tag="g1")
            nc.sync.dma_start(g1[:], g1_ext.ap().rearrange("(c p) -> p c", p=128))
            b1 = constp.tile([128, CT], F32, tag="b1")
            nc.sync.dma_start(b1[:], b1_ext.ap().rearrange("(c p) -> p c", p=128))
            g2h = constp.tile([64, H], F32, tag="g2h")
            nc.sync.dma_start(g2h[:], g2_ext.ap().rearrange("(h p) -> p h", p=64))
            b2h = constp.tile([64, H], F32, tag="b2h")
            nc.sync.dma_start(b2h[:], b2_ext.ap().rearrange("(h p) -> p h", p=64))
            bp = constp.tile([128, CT], F32, tag="bp")
            nc.sync.dma_start(bp[:], bp_ext.ap().rearrange("(c p) -> p c", p=128))

            qT = [qTp.tile([128, R], F16, tag=f"qT{p}", name=f"qT{p}") for p in range(H // 2)]
            opair = [
                op_.tile([128, R], F16, tag=f"op{p}", name=f"op{p}")
                for p in range(H // 2)
            ]

            # ---------------- Phase 1: LN1 + K/V/Q projections ----------
            with (
                tc.tile_pool(name="xn", bufs=1) as xnp,
                tc.tile_pool(name="w1", bufs=1) as w1p,
                tc.tile_pool(name="tmp1", bufs=3) as tmp1p,
                tc.tile_pool(name="small1", bufs=1) as small1p,
                tc.tile_pool(name="ev1", bufs=4) as ev1p,
                tc.tile_pool(name="xt", bufs=1) as xtp,
                tc.tile_pool(name="ps_stat", bufs=1, space="PSUM") as ps_stat,
                tc.tile_pool(name="ps_bc", bufs=1, space="PSUM") as ps_bc,
                tc.tile_pool(name="ps_mm", bufs=2, space="PSUM") as ps_mm,
            ):
                xt = []
                for c in range(CT):
                    t = xtp.tile([128, R], F16, tag=f"xt{c}")
                    nc.sync.dma_start(t[:], xT_ext[128 * c : 128 * (c + 1), :])
                    xt.append(t)

                # resident fp16 weights
                wqk_sb = []
                wv_sb = []
                for c in range(CT):
                    t = w1p.tile([128, 2 * C], F16, tag=f"wqk{c}", name=f"wqk{c}")
                    nc.sync.dma_start(t[:], wqk_ext[128 * c : 128 * (c + 1), :])
                    wqk_sb.append(t)
                    t = w1p.tile([128, C], F16, tag=f"wv{c}", name=f"wv{c}")
                    nc.sync.dma_start(t[:], wv_ext[128 * c : 128 * (c + 1), :])
                    wv_sb.append(t)

                # LN1 stats: sums of x and x^2 over C (partition axis)
                sx_ps = ps_stat.tile([1, R], F32, tag="sx")
                sq_ps = ps_stat.tile([1, R], F32, tag="sq")
                for c in range(CT):
                    nc.tensor.matmul(
                        sx_ps[:], ones_col[:], xt[c][:], start=(c == 0), stop=(c == CT - 1)
                    )
                for c in range(CT):
                    xsq = tmp1p.tile([128, R], F16, tag="xsq")
                    if c % 2 == 0:
                        nc.scalar.activation(xsq[:], xt[c][:], AF.Square)
                    else:
                        nc.vector.tensor_tensor(xsq[:], xt[c][:], xt[c][:], op=ALU.mult)
                    nc.tensor.matmul(
                        sq_ps[:], ones_col[:], xsq[:], start=(c == 0), stop=(c == CT - 1)
                    )

                mu = small1p.tile([1, R], F16, tag="mu")
                nc.vector.tensor_scalar_mul(mu[:], sx_ps[:], 1.0 / C)
                m2 = small1p.tile([1, R], F32, tag="m2")
                nc.vector.tensor_scalar_mul(m2[:], sq_ps[:], 1.0 / C)
                musq = small1p.tile([1, R], F32, tag="musq")
                nc.vector.tensor_tensor(musq[:], mu[:], mu[:], op=ALU.mult)
                var = small1p.tile([1, R], F32, tag="var")
                nc.vector.tensor_tensor(var[:], m2[:], musq[:], op=ALU.subtract)
                lv = small1p.tile([1, R], F32, tag="lv")
                nc.scalar.activation(lv[:], var[:], AF.Ln, bias=eps_t[:])
                rsig = small1p.tile([1, R], F16, tag="rsig")
                nc.scalar.activation(rsig[:], lv[:], AF.Exp, scale=-0.5)

                bmu_ps = ps_bc.tile([128, R], F32, tag="bmu")
                nc.tensor.matmul(bmu_ps[:], ones_row[:], mu[:], start=True, stop=True)
                brs_ps = ps_bc.tile([128, R], F32, tag="brs")
                nc.tensor.matmul(brs_ps[:], ones_row[:], rsig[:], start=True, stop=True)

                xn = []
                for c in range(CT):
                    t1 = tmp1p.tile([128, R], F32, tag="lt1")
                    nc.vector.tensor_tensor(t1[:], xt[c][:], bmu_ps[:], op=ALU.subtract)
                    t2 = tmp1p.tile([128, R], F32, tag="lt2")
                    nc.vector.tensor_tensor(t2[:], t1[:], brs_ps[:], op=ALU.mult)
                    t3 = xnp.tile([128, R], F16, tag=f"xn{c}")
                    nc.vector.tensor_scalar(
                        t3[:], t2[:], g1[:, c : c + 1], b1[:, c : c + 1],
                        op0=ALU.mult, op1=ALU.add,
                    )
                    xn.append(t3)

                # k^T = w_k^T @ xn^T -> [C, R] feature-major, gathered in
                # two halves so comms start as early as possible
                for half in range(2):
                    for kfh in range(CT // 2):
                        kf = half * (CT // 2) + kfh
                        kps = ps_mm.tile([128, R], F32, tag="kvps")
                        for c in range(CT):
                            nc.tensor.matmul(
                                kps[:],
                                wqk_sb[c][:, C + 128 * kf : C + 128 * (kf + 1)],
                                xn[c][:],
                                start=(c == 0),
                                stop=(c == CT - 1),
                            )
                        ksb = ev1p.tile([128, R], F16, tag="kev")
                        nc.scalar.copy(ksb[:], kps[:])
                        nc.sync.dma_start(
                            k_in[half][128 * kfh : 128 * (kfh + 1), :], ksb[:]
                        )
                    nc.gpsimd.collective_compute(
                        "AllGather",
                        ALU.bypass,
                        ins=[k_in[half][:]],
                        outs=[k_out[half][:]],
                        replica_groups=[[0, 1, 2, 3], [4, 5, 6, 7]],
                    )

                # v = xn @ w_v -> [R, H, 65] row-major with ones cols, gather
                for rt in range(R // 128):
                    for vf in range(2):
                        vps = ps_mm.tile([128, R], F32, tag="kvps")
                        for c in range(CT):
                            nc.tensor.matmul(
                                vps[:],
                                xn[c][:, 128 * rt : 128 * (rt + 1)],
                                wv_sb[c][:, 512 * vf : 512 * (vf + 1)],
                                start=(c == 0),
                                stop=(c == CT - 1),
                            )
                        vsb = ev1p.tile([128, 8, HD + 1], F16, tag="vev")
                        nc.scalar.copy(
                            vsb[:, :, 0:HD],
                            vps[:].rearrange("p (h d) -> p h d", d=HD),
                        )
                        nc.vector.memset(vsb[:, :, HD : HD + 1], 1.0)
                        nc.sync.dma_start(
                            v_in[128 * rt : 128 * (rt + 1), 8 * vf : 8 * (vf + 1), :],
                            vsb[:],
                        )

                nc.gpsimd.collective_compute(
                    "AllGather",
                    ALU.bypass,
                    ins=[v_in[:]],
                    outs=[v_out[:]],
                    replica_groups=[[0, 1, 2, 3], [4, 5, 6, 7]],
                )

                # q^T = w_q^T @ xn^T -> [C, R] (overlaps the all-gathers)
                for qf in range(CT):
                    qps = ps_mm.tile([128, R], F32, tag="kvps")
                    for c in range(CT):
                        nc.tensor.matmul(
                            qps[:],
                            wqk_sb[c][:, 128 * qf : 128 * (qf + 1)],
                            xn[c][:],
                            start=(c == 0),
                            stop=(c == CT - 1),
                        )
                    nc.scalar.copy(qT[qf][:], qps[:])

            # ---------------- Phase 2: attention -------------------------
            # All score/exp work first (depends only on the K gather), then
            # all AV work (needs the V gather) — keeps the PE busy through
            # the V all-gather window.  att slots throttle the lookahead.
            with (
                tc.tile_pool(name="kpair", bufs=2) as kpp,
                tc.tile_pool(name="vpair", bufs=3) as vpp,
                tc.tile_pool(name="att", bufs=48) as attp,
                tc.tile_pool(name="rec", bufs=4) as recp,
                tc.tile_pool(name="oh", bufs=4) as ohp,
                tc.tile_pool(name="ps_sc", bufs=2, space="PSUM") as ps_sc,
                tc.tile_pool(name="ps_av", bufs=2, space="PSUM") as ps_av,
            ):
                NP = H // 2
                att = [None] * NP

                for p in range(NP):
                    kpair = kpp.tile([128, GROUP * R], F16, tag="kpair", name=f"kp{p}")
                    half, ph = divmod(p, CT // 2)
                    nc.sync.dma_start(
                        kpair[:].rearrange("p (r n) -> p r n", r=GROUP),
                        k_out[half][:, 128 * ph : 128 * (ph + 1), :].rearrange(
                            "r p n -> p r n"
                        ),
                    )
                    att[p] = [[], []]
                    for g in range(NKC // 2):
                        scp = [None, None]
                        for hi in range(2):
                            base = 64 * hi
                            scp[hi] = ps_sc.tile(
                                [128, 1024], F32, tag="scps", name=f"sc{p}_{g}_{hi}"
                            )
                            for cc in range(2):
                                j = 2 * g + cc
                                nc.tensor.matmul(
                                    scp[hi][:, 512 * cc : 512 * (cc + 1)],
                                    kpair[base : base + 64, 128 * j : 128 * (j + 1)],
                                    qT[p][base : base + 64, :],
                                    start=True,
                                    stop=True,
                                )
                        for hi in range(2):
                            at = attp.tile([128, 1024], F16, tag="att")
                            nc.scalar.activation(at[:], scp[hi][:], AF.Exp, scale=SCALE)
                            att[p][hi].append(at)

                for p in range(NP):
                    vpair = vpp.tile(
                        [128, NKC, 2 * (HD + 1)], F16, tag="vpair", name=f"vp{p}"
                    )
                    nc.sync.dma_start(
                        vpair[:],
                        v_out[:]
                        .rearrange("r (j p) h d -> p (r j) (h d)", p=128)[
                            :, :, (HD + 1) * 2 * p : (HD + 1) * 2 * (p + 1)
                        ],
                    )
                    for hi in range(2):
                        h = 2 * p + hi
                        avps = ps_av.tile([HD + 1, R], F32, tag="avps", name=f"av{h}")
                        for j in range(NKC):
                            nc.tensor.matmul(
                                avps[:],
                                vpair[:, j, (HD + 1) * hi : (HD + 1) * (hi + 1)],
                                att[p][hi][j // 2][:, 512 * (j % 2) : 512 * (j % 2 + 1)],
                                start=(j == 0),
                                stop=(j == NKC - 1),
                            )
                        den = recp.tile([1, R], F32, tag="den")
                        nc.vector.tensor_copy(den[:], avps[HD : HD + 1, :])
                        recf = recp.tile([1, R], F32, tag="recf")
                        with nc.allow_low_precision(reason="softmax denom bcast"):
                            nc.vector.reciprocal_approx_fast(recf[:], den[:])
                        rec = recp.tile([1, R], F16, tag="rec")
                        nc.vector.tensor_copy(rec[:], recf[:])
                        oh = ohp.tile([64, R], F16, tag="oh")
                        nc.vector.tensor_copy(oh[:], avps[0:HD, :])
                        # broadcast 1/denom into the (now free) avps bank
                        nc.tensor.matmul(
                            avps[0:HD, :], ones_row[:, 0:HD], rec[:],
                            start=True, stop=True,
                        )
                        nc.vector.tensor_tensor(oh[:], oh[:], avps[0:HD, :], op=ALU.mult)
                        # pack the two heads of a pair into one 128-partition
                        # tile (SBUF->SBUF DMA is the only partition shifter)
                        nc.sync.dma_start(opair[p][64 * hi : 64 * (hi + 1), :], oh[:])
                    # LN2 statistics accumulate while attention proceeds
                    nc.tensor.matmul(
                        s2_ps[:], ones_col[:], opair[p][:],
                        start=(p == 0), stop=(p == NP - 1),
                    )
                    osq = recp.tile([128, R], F16, tag="osq")
                    nc.vector.tensor_tensor(osq[:], opair[p][:], opair[p][:], op=ALU.mult)
                    nc.tensor.matmul(
                        q2_ps[:], ones_col[:], osq[:],
                        start=(p == 0), stop=(p == NP - 1),
                    )
                    att[p] = None

            # ---------------- Phase 3: LN2 + projection ------------------
            with (
                tc.tile_pool(name="wp", bufs=1) as wpp,
                tc.tile_pool(name="tmp2", bufs=4) as tmp2p,
                tc.tile_pool(name="small2", bufs=1) as small2p,
                tc.tile_pool(name="ln2o", bufs=1) as ln2op,
                tc.tile_pool(name="yev", bufs=3) as yevp,
                tc.tile_pool(name="ps_bc2", bufs=1, space="PSUM") as ps_bc2,
                tc.tile_pool(name="ps_y", bufs=4, space="PSUM") as ps_y,
            ):
                wp_sb = []
                for p in range(H // 2):
                    t = wpp.tile([128, C], F16, tag=f"wp{p}", name=f"wpp{p}")
                    nc.sync.dma_start(t[:], wp_ext[128 * p : 128 * (p + 1), :])
                    wp_sb.append(t)

                mu2 = small2p.tile([1, R], F16, tag="mu2")
                nc.vector.tensor_scalar_mul(mu2[:], s2_ps[:], 1.0 / C)
                m22 = small2p.tile([1, R], F32, tag="m22")
                nc.vector.tensor_scalar_mul(m22[:], q2_ps[:], 1.0 / C)
                musq2 = small2p.tile([1, R], F32, tag="musq2")
                nc.vector.tensor_tensor(musq2[:], mu2[:], mu2[:], op=ALU.mult)
                var2 = small2p.tile([1, R], F32, tag="var2")
                nc.vector.tensor_tensor(var2[:], m22[:], musq2[:], op=ALU.subtract)
                lv2 = small2p.tile([1, R], F32, tag="lv2")
                nc.scalar.activation(lv2[:], var2[:], AF.Ln, bias=eps_t[:])
                rsig2 = small2p.tile([1, R], F16, tag="rsig2")
                nc.scalar.activation(rsig2[:], lv2[:], AF.Exp, scale=-0.5)

                bmu2_ps = ps_bc2.tile([128, R], F32, tag="bmu2")
                nc.tensor.matmul(
                    bmu2_ps[:], ones_row[:], mu2[:], start=True, stop=True
                )
                brs2_ps = ps_bc2.tile([128, R], F32, tag="brs2")
                nc.tensor.matmul(
                    brs2_ps[:], ones_row[:], rsig2[:], start=True, stop=True
                )

                g2p = constp.tile([128, H // 2], F32, tag="g2p")
                nc.sync.dma_start(g2p[:], g2_ext.ap().rearrange("(p q) -> q p", q=128))
                b2p = constp.tile([128, H // 2], F32, tag="b2p")
                nc.sync.dma_start(b2p[:], b2_ext.ap().rearrange("(p q) -> q p", q=128))

                ln2o = []
                for p in range(H // 2):
                    t1 = tmp2p.tile([128, R], F32, tag="l2t1")
                    nc.vector.tensor_tensor(
                        t1[:], opair[p][:], bmu2_ps[:], op=ALU.subtract
                    )
                    t2 = tmp2p.tile([128, R], F32, tag="l2t2")
                    nc.vector.tensor_tensor(t2[:], t1[:], brs2_ps[:], op=ALU.mult)
                    t3 = ln2op.tile([128, R], F16, tag=f"ln2o{p}", name=f"ln2o{p}")
                    nc.vector.tensor_scalar(
                        t3[:], t2[:], g2p[:, p : p + 1], b2p[:, p : p + 1],
                        op0=ALU.mult, op1=ALU.add,
                    )
                    ln2o.append(t3)

                # h-outer projection: matmuls for pair p start as soon as
                # ln2o[p] exists, overlapping the LN2 apply chain
                for half in range(2):
                    yps_t = [
                        ps_y.tile([128, R], F32, tag="yps", name=f"yps{half}_{i}")
                        for i in range(4)
                    ]
                    for p in range(H // 2):
                        for i in range(4):
                            of = 4 * half + i
                            nc.tensor.matmul(
                                yps_t[i][:],
                                wp_sb[p][:, 128 * of : 128 * (of + 1)],
                                ln2o[p][:],
                                start=(p == 0),
                                stop=(p == H // 2 - 1),
                            )
                    for i in range(4):
                        of = 4 * half + i
                        ysb = yevp.tile([128, R], F32, tag="yev")
                        nc.scalar.activation(
                            ysb[:], yps_t[i][:], AF.Identity, bias=bp[:, of : of + 1]
                        )
                        nc.sync.dma_start(
                            out_ext[128 * of : 128 * (of + 1), :], ysb[:]
                        )

    nc.compile()
    return nc


def kernel(x, ln1_g, ln1_b, w_qk, w_v, ln2_g, ln2_b, w_proj, b_proj):
    global _cached_nc, last_exec_time_ns
    if _cached_nc is None:
        _cached_nc = _build()
    nc = _cached_nc

    xr = np.asarray(x, np.float32).reshape(B * N, C).astype(np.float16)
    shared = {
        "w_qk": np.asarray(w_qk, np.float32).astype(np.float16),
        "w_v": np.asarray(w_v, np.float32).astype(np.float16),
        "w_proj": np.asarray(w_proj, np.float32).astype(np.float16),
        "ln1_g": np.ascontiguousarray(ln1_g, np.float32),
        "ln1_b": np.ascontiguousarray(ln1_b, np.float32),
        "ln2_g": np.ascontiguousarray(ln2_g, np.float32),
        "ln2_b": np.ascontiguousarray(ln2_b, np.float32),
        "b_proj": np.ascontiguousarray(b_proj, np.float32),
    }
    in_maps = []
    for i in range(NCORES):
        xT_i = np.ascontiguousarray(xr[R * i : R * (i + 1), :].T)
        in_maps.append({"xT": xT_i, **shared})

    res = run_bass_kernel_spmd(nc, in_maps, core_ids=list(range(NCORES)))
    last_exec_time_ns = res.exec_time_ns

    y = np.empty((B * N, C), np.float32)
    for i in range(NCORES):
        y[R * i : R * (i + 1), :] = res.results[i]["out"].T
    return y.reshape(B, N, C)


# revision 16
# speedup vs baseline: 1.9620x; 1.0112x over previous
"""Distributed Trainium2 kernel for the dense-transformer attention block:

    xn = LN(x); q,k = xn @ w_qk; v = xn @ w_v
    out = softmax(q k^T / sqrt(hd)) v ; out = LN(out) @ w_proj + b_proj

Sharding: the (B=2, N=2048) token axis is flattened to 4096 rows, 512 per
core (cores 0-3 own batch 0, cores 4-7 batch 1).  Each core runs LN1 and
the K/V projections on its rows, all-gathers K^T and V (fp16) inside its
4-core batch group, computes all 16 heads of attention for its own 512
query rows, then LN2 and the output projection — every FLOP except the
gathers is done exactly once across the chip.

On-chip data lives in a transposed ("feature on partition") layout: the
host feeds x^T per core and transposes the returned y^T back (free).
Matmul operands are fp16 (full PE rate + fast weight load); PSUM
accumulation and LayerNorm statistics stay fp32.  Softmax denominators
come free from a ones-column appended to V (row 64 of the AV output);
no max-subtraction is needed because scores are ~N(0,1).
"""

import numpy as np

import concourse.bass as bass
import concourse.mybir as mybir
import concourse.tile as tile
from concourse import bacc
from concourse.bass_utils import run_bass_kernel_spmd

B, N, C = 2, 2048, 1024
H, HD = 16, 64
NCORES = 8
R = (B * N) // NCORES  # 512 rows per core
GROUP = NCORES // B  # 4 cores per batch
NKC = (GROUP * R) // 128  # 16 k-chunks of 128
EPS = 1e-5
SCALE = HD**-0.5

F32 = mybir.dt.float32
F16 = mybir.dt.float16
AF = mybir.ActivationFunctionType
ALU = mybir.AluOpType

CT = C // 128  # 8 C tiles

last_exec_time_ns = None
_cached_nc = None


def _build():
    nc = bacc.Bacc("TRN2", target_bir_lowering=False, debug=False, num_devices=NCORES)

    xT_ext = nc.dram_tensor("xT", [C, R], F16, kind="ExternalInput")
    wqk_ext = nc.dram_tensor("w_qk", [C, 2 * C], F16, kind="ExternalInput")
    wv_ext = nc.dram_tensor("w_v", [C, C], F16, kind="ExternalInput")
    wp_ext = nc.dram_tensor("w_proj", [C, C], F16, kind="ExternalInput")
    g1_ext = nc.dram_tensor("ln1_g", [C], F32, kind="ExternalInput")
    b1_ext = nc.dram_tensor("ln1_b", [C], F32, kind="ExternalInput")
    g2_ext = nc.dram_tensor("ln2_g", [C], F32, kind="ExternalInput")
    b2_ext = nc.dram_tensor("ln2_b", [C], F32, kind="ExternalInput")
    bp_ext = nc.dram_tensor("b_proj", [C], F32, kind="ExternalInput")
    out_ext = nc.dram_tensor("out", [C, R], F32, kind="ExternalOutput")

    # k^T split in two halves so the first gather starts earlier
    k_in = [
        nc.dram_tensor(f"k_in{i}", [C // 2, R], F16) for i in range(2)
    ]  # k^T, feature-major
    k_out = [nc.dram_tensor(f"k_out{i}", [GROUP, C // 2, R], F16) for i in range(2)]
    # v + ones col per head, split in two halves (heads 0-7 / 8-15)
    v_in = [nc.dram_tensor(f"v_in{i}", [R, H // 2, HD + 1], F16) for i in range(2)]
    v_out = [
        nc.dram_tensor(f"v_out{i}", [GROUP, R, H // 2, HD + 1], F16) for i in range(2)
    ]

    with tile.TileContext(nc) as tc:
        with (
            tc.tile_pool(name="const", bufs=1) as constp,
            tc.tile_pool(name="qT", bufs=1) as qTp,
            tc.tile_pool(name="o", bufs=1) as op_,
            tc.tile_pool(name="acc", bufs=1) as accp,
        ):
            s2acc = accp.tile([1, R], F32, tag="s2acc")
            nc.vector.memset(s2acc[:], 0.0)
            q2acc = accp.tile([1, R], F32, tag="q2acc")
            nc.vector.memset(q2acc[:], 0.0)
            ones_col = constp.tile([128, 1], F16, tag="onesc")
            nc.vector.memset(ones_col[:], 1.0)
            ones_row = constp.tile([1, 128], F16, tag="onesr")
            nc.vector.memset(ones_row[:], 1.0)
            eps_t = constp.tile([1, 1], F32, tag="epsc")
            nc.sync.dma_start(
                eps_t[:], nc.inline_tensor(np.full((1, 1), EPS, np.float32), "epsc").ap()
            )
            g1 = constp.tile([128, CT], F32, tag="g1")
            nc.sync.dma_start(g1[:], g1_ext.ap().rearrange("(c p) -> p c", p=128))
            b1 = constp.tile([128, CT], F32, tag="b1")
            nc.sync.dma_start(b1[:], b1_ext.ap().rearrange("(c p) -> p c", p=128))
            g2h = constp.tile([64, H], F32, tag="g2h")
            nc.sync.dma_start(g2h[:], g2_ext.ap().rearrange("(h p) -> p h", p=64))
            b2h = constp.tile([64, H], F32, tag="b2h")
            nc.sync.dma_start(b2h[:], b2_ext.ap().rearrange("(h p) -> p h", p=64))
            bp = constp.tile([128, CT], F32, tag="bp")
            nc.sync.dma_start(bp[:], bp_ext.ap().rearrange("(c p) -> p c", p=128))

            qT = [qTp.tile([128, R], F16, tag=f"qT{p}", name=f"qT{p}") for p in range(H // 2)]
            opair = [
                op_.tile([128, R], F16, tag=f"op{p}", name=f"op{p}")
                for p in range(H // 2)
            ]

            # ---------------- Phase 1: LN1 + K/V/Q projections ----------
            with (
                tc.tile_pool(name="xn", bufs=1) as xnp,
                tc.tile_pool(name="w1", bufs=1) as w1p,
                tc.tile_pool(name="tmp1", bufs=3) as tmp1p,
                tc.tile_pool(name="small1", bufs=1) as small1p,
                tc.tile_pool(name="ev1", bufs=4) as ev1p,
                tc.tile_pool(name="xt", bufs=1) as xtp,
                tc.tile_pool(name="ps_stat", bufs=1, space="PSUM") as ps_stat,
                tc.tile_pool(name="ps_bc", bufs=1, space="PSUM") as ps_bc,
                tc.tile_pool(name="ps_mm", bufs=3, space="PSUM") as ps_mm,
            ):
                xt = []
                for c in range(CT):
                    t = xtp.tile([128, R], F16, tag=f"xt{c}")
                    nc.sync.dma_start(t[:], xT_ext[128 * c : 128 * (c + 1), :])
                    xt.append(t)

                # resident fp16 weights; K-projection slabs first so the
                # k^T matmuls (feeding the first all-gather) never wait
                wk_sb = []
                wq_sb = []
                wv_sb = []
                for c in range(CT):
                    t = w1p.tile([128, C], F16, tag=f"wk{c}", name=f"wk{c}")
                    nc.sync.dma_start(t[:], wqk_ext[128 * c : 128 * (c + 1), C:])
                    wk_sb.append(t)
                for c in range(CT):
                    t = w1p.tile([128, C], F16, tag=f"wv{c}", name=f"wv{c}")
                    nc.sync.dma_start(t[:], wv_ext[128 * c : 128 * (c + 1), :])
                    wv_sb.append(t)
                for c in range(CT):
                    t = w1p.tile([128, C], F16, tag=f"wq{c}", name=f"wq{c}")
                    nc.sync.dma_start(t[:], wqk_ext[128 * c : 128 * (c + 1), 0:C])
                    wq_sb.append(t)

                # LN1 stats: sums of x and x^2 over C (partition axis)
                sx_ps = ps_stat.tile([1, R], F32, tag="sx")
                sq_ps = ps_stat.tile([1, R], F32, tag="sq")
                for c in range(CT):
                    nc.tensor.matmul(
                        sx_ps[:], ones_col[:], xt[c][:], start=(c == 0), stop=(c == CT - 1)
                    )
                for c in range(CT):
                    xsq = tmp1p.tile([128, R], F16, tag="xsq")
                    if c % 2 == 0:
                        nc.scalar.activation(xsq[:], xt[c][:], AF.Square)
                    else:
                        nc.vector.tensor_tensor(xsq[:], xt[c][:], xt[c][:], op=ALU.mult)
                    nc.tensor.matmul(
                        sq_ps[:], ones_col[:], xsq[:], start=(c == 0), stop=(c == CT - 1)
                    )

                mu = small1p.tile([1, R], F16, tag="mu")
                nc.vector.tensor_scalar_mul(mu[:], sx_ps[:], 1.0 / C)
                m2 = small1p.tile([1, R], F32, tag="m2")
                nc.vector.tensor_scalar_mul(m2[:], sq_ps[:], 1.0 / C)
                musq = small1p.tile([1, R], F32, tag="musq")
                nc.vector.tensor_tensor(musq[:], mu[:], mu[:], op=ALU.mult)
                var = small1p.tile([1, R], F32, tag="var")
                nc.vector.tensor_tensor(var[:], m2[:], musq[:], op=ALU.subtract)
                lv = small1p.tile([1, R], F32, tag="lv")
                nc.scalar.activation(lv[:], var[:], AF.Ln, bias=eps_t[:])
                rsig = small1p.tile([1, R], F16, tag="rsig")
                nc.scalar.activation(rsig[:], lv[:], AF.Exp, scale=-0.5)

                bmu_ps = ps_bc.tile([128, R], F32, tag="bmu")
                nc.tensor.matmul(bmu_ps[:], ones_row[:], mu[:], start=True, stop=True)
                brs_ps = ps_bc.tile([128, R], F32, tag="brs")
                nc.tensor.matmul(brs_ps[:], ones_row[:], rsig[:], start=True, stop=True)

                xn = []
                for c in range(CT):
                    t1 = tmp1p.tile([128, R], F32, tag="lt1")
                    nc.vector.tensor_tensor(t1[:], xt[c][:], bmu_ps[:], op=ALU.subtract)
                    t2 = tmp1p.tile([128, R], F32, tag="lt2")
                    nc.vector.tensor_tensor(t2[:], t1[:], brs_ps[:], op=ALU.mult)
                    t3 = xnp.tile([128, R], F16, tag=f"xn{c}")
                    nc.vector.tensor_scalar(
                        t3[:], t2[:], g1[:, c : c + 1], b1[:, c : c + 1],
                        op0=ALU.mult, op1=ALU.add,
                    )
                    xn.append(t3)

                # k^T and v computed and gathered in interleaved halves:
                # AG(k0) AG(v0) AG(k1) AG(v1), so score work for pairs 0-3
                # overlaps AG(v0), AV work for pairs 0-3 overlaps AG(k1), ...
                for half in range(2):
                    for kfh in range(CT // 2):
                        kf = half * (CT // 2) + kfh
                        kps = ps_mm.tile([128, R], F32, tag="kvps")
                        for c in range(CT):
                            nc.tensor.matmul(
                                kps[:],
                                wk_sb[c][:, 128 * kf : 128 * (kf + 1)],
                                xn[c][:],
                                start=(c == 0),
                                stop=(c == CT - 1),
                            )
                        ksb = ev1p.tile([128, R], F16, tag="kev")
                        nc.scalar.copy(ksb[:], kps[:])
                        nc.sync.dma_start(
                            k_in[half][128 * kfh : 128 * (kfh + 1), :], ksb[:]
                        )
                    nc.gpsimd.collective_compute(
                        "AllGather",
                        ALU.bypass,
                        ins=[k_in[half][:]],
                        outs=[k_out[half][:]],
                        replica_groups=[[0, 1, 2, 3], [4, 5, 6, 7]],
                    )
                    # v half: heads 8*half .. 8*half+8
                    for rt in range(R // 128):
                        vps = ps_mm.tile([128, R], F32, tag="kvps")
                        for c in range(CT):
                            nc.tensor.matmul(
                                vps[:],
                                xn[c][:, 128 * rt : 128 * (rt + 1)],
                                wv_sb[c][:, 512 * half : 512 * (half + 1)],
                                start=(c == 0),
                                stop=(c == CT - 1),
                            )
                        vsb = ev1p.tile([128, 8, HD + 1], F16, tag="vev")
                        nc.scalar.copy(
                            vsb[:, :, 0:HD],
                            vps[:].rearrange("p (h d) -> p h d", d=HD),
                        )
                        nc.vector.memset(vsb[:, :, HD : HD + 1], 1.0)
                        nc.sync.dma_start(
                            v_in[half][128 * rt : 128 * (rt + 1), :, :], vsb[:]
                        )
                    nc.gpsimd.collective_compute(
                        "AllGather",
                        ALU.bypass,
                        ins=[v_in[half][:]],
                        outs=[v_out[half][:]],
                        replica_groups=[[0, 1, 2, 3], [4, 5, 6, 7]],
                    )

                # q^T = w_q^T @ xn^T -> [C, R] (overlaps the all-gathers)
                for qf in range(CT):
                    qps = ps_mm.tile([128, R], F32, tag="kvps")
                    for c in range(CT):
                        nc.tensor.matmul(
                            qps[:],
                            wq_sb[c][:, 128 * qf : 128 * (qf + 1)],
                            xn[c][:],
                            start=(c == 0),
                            stop=(c == CT - 1),
                        )
                    nc.scalar.copy(qT[qf][:], qps[:])

            # ---------------- Phase 2: attention -------------------------
            # Wave w covers head-pairs 4w..4w+3: scores depend on AG(k_w),
            # AV on AG(v_w); emission order (sc wave0, av wave0, sc wave1,
            # av wave1) keeps the PE busy through every gather window.
            with (
                tc.tile_pool(name="kpair", bufs=2) as kpp,
                tc.tile_pool(name="vpair", bufs=3) as vpp,
                tc.tile_pool(name="att", bufs=32) as attp,
                tc.tile_pool(name="rec", bufs=6) as recp,
                tc.tile_pool(name="oh", bufs=4) as ohp,
                tc.tile_pool(name="ps_sc", bufs=2, space="PSUM") as ps_sc,
                tc.tile_pool(name="ps_av", bufs=2, space="PSUM") as ps_av,
                tc.tile_pool(name="ps_nb", bufs=1, space="PSUM") as ps_nb,
                tc.tile_pool(name="ps_srot", bufs=1, space="PSUM") as ps_srot,
            ):
                NP = H // 2
                att = [None] * NP

                def scores_pair(p):
                    kpair = kpp.tile([128, GROUP * R], F16, tag="kpair", name=f"kp{p}")
                    half, ph = divmod(p, CT // 2)
                    nc.sync.dma_start(
                        kpair[:].rearrange("p (r n) -> p r n", r=GROUP),
                        k_out[half][:, 128 * ph : 128 * (ph + 1), :].rearrange(
                            "r p n -> p r n"
                        ),
                    )
                    att[p] = [[], []]
                    for g in range(NKC // 2):
                        scp = [None, None]
                        for hi in range(2):
                            scp[hi] = ps_sc.tile(
                                [128, 1024], F32, tag="scps", name=f"sc{p}_{g}_{hi}"
                            )
                        for cc in range(2):
                            j = 2 * g + cc
                            for hi in range(2):
                                base = 64 * hi
                                nc.tensor.matmul(
                                    scp[hi][:, 512 * cc : 512 * (cc + 1)],
                                    kpair[base : base + 64, 128 * j : 128 * (j + 1)],
                                    qT[p][base : base + 64, :],
                                    start=True,
                                    stop=True,
                                )
                        for hi in range(2):
                            at = attp.tile([128, 1024], F16, tag="att")
                            nc.scalar.activation(at[:], scp[hi][:], AF.Exp, scale=SCALE)
                            att[p][hi].append(at)

                def av_pair(p):
                    half = p // (CT // 2)
                    vpair = vpp.tile(
                        [128, NKC, 2 * (HD + 1)], F16, tag="vpair", name=f"vp{p}"
                    )
                    ph = p % (CT // 2)
                    nc.sync.dma_start(
                        vpair[:],
                        v_out[half][:]
                        .rearrange("r (j p) h d -> p (r j) (h d)", p=128)[
                            :, :, (HD + 1) * 2 * ph : (HD + 1) * 2 * (ph + 1)
                        ],
                    )
                    for hi in range(2):
                        h = 2 * p + hi
                        avps = ps_av.tile([HD + 1, R], F32, tag="avps", name=f"av{h}")
                        for j in range(NKC):
                            nc.tensor.matmul(
                                avps[:],
                                vpair[:, j, (HD + 1) * hi : (HD + 1) * (hi + 1)],
                                att[p][hi][j // 2][:, 512 * (j % 2) : 512 * (j % 2 + 1)],
                                start=(j == 0),
                                stop=(j == NKC - 1),
                            )
                        # two quick copies release the AV bank, the rest of
                        # the softmax-normalization chain runs off-psum
                        oh = ohp.tile([64, R], F16, tag="oh")
                        nc.vector.tensor_copy(oh[:], avps[0:HD, :])
                        den = recp.tile([1, R], F32, tag="den")
                        nc.vector.tensor_copy(den[:], avps[HD : HD + 1, :])
                        recf = recp.tile([1, R], F32, tag="recf")
                        with nc.allow_low_precision(reason="softmax denom bcast"):
                            nc.vector.reciprocal_approx_fast(recf[:], den[:])
                        rec = recp.tile([1, R], F16, tag="rec")
                        nc.vector.tensor_copy(rec[:], recf[:])
                        nbps = ps_nb.tile([64, R], F32, tag="nbps", name=f"nb{h}")
                        nc.tensor.matmul(
                            nbps[:], ones_row[:, 0:64], rec[:], start=True, stop=True
                        )
                        nc.vector.tensor_tensor(oh[:], oh[:], nbps[:], op=ALU.mult)
                        nc.sync.dma_start(opair[p][64 * hi : 64 * (hi + 1), :], oh[:])
                    # LN2 statistics accumulate (SBUF) while attention runs
                    s2p = ps_srot.tile([1, R], F32, tag="s2p", name=f"s2p{p}")
                    nc.tensor.matmul(
                        s2p[:], ones_col[:], opair[p][:], start=True, stop=True
                    )
                    nc.vector.tensor_tensor(s2acc[:], s2acc[:], s2p[:], op=ALU.add)
                    osq = recp.tile([128, R], F16, tag="osq")
                    nc.vector.tensor_tensor(osq[:], opair[p][:], opair[p][:], op=ALU.mult)
                    q2p = ps_srot.tile([1, R], F32, tag="s2p", name=f"q2p{p}")
                    nc.tensor.matmul(q2p[:], ones_col[:], osq[:], start=True, stop=True)
                    nc.vector.tensor_tensor(q2acc[:], q2acc[:], q2p[:], op=ALU.add)
                    att[p] = None

                for wave in range(2):
                    for p in range(4 * wave, 4 * wave + 4):
                        scores_pair(p)
                    for p in range(4 * wave, 4 * wave + 4):
                        av_pair(p)

            # ---------------- Phase 3: LN2 + projection ------------------
            with (
                tc.tile_pool(name="wp", bufs=1) as wpp,
                tc.tile_pool(name="tmp2", bufs=4) as tmp2p,
                tc.tile_pool(name="small2", bufs=1) as small2p,
                tc.tile_pool(name="ln2o", bufs=1) as ln2op,
                tc.tile_pool(name="yev", bufs=3) as yevp,
                tc.tile_pool(name="ps_bc2", bufs=1, space="PSUM") as ps_bc2,
                tc.tile_pool(name="ps_y", bufs=4, space="PSUM") as ps_y,
            ):
                wp_sb = []
                for p in range(H // 2):
                    t = wpp.tile([128, C], F16, tag=f"wp{p}", name=f"wpp{p}")
                    nc.sync.dma_start(t[:], wp_ext[128 * p : 128 * (p + 1), :])
                    wp_sb.append(t)

                mu2 = small2p.tile([1, R], F16, tag="mu2")
                nc.vector.tensor_scalar_mul(mu2[:], s2acc[:], 1.0 / C)
                m22 = small2p.tile([1, R], F32, tag="m22")
                nc.vector.tensor_scalar_mul(m22[:], q2acc[:], 1.0 / C)
                musq2 = small2p.tile([1, R], F32, tag="musq2")
                nc.vector.tensor_tensor(musq2[:], mu2[:], mu2[:], op=ALU.mult)
                var2 = small2p.tile([1, R], F32, tag="var2")
                nc.vector.tensor_tensor(var2[:], m22[:], musq2[:], op=ALU.subtract)
                lv2 = small2p.tile([1, R], F32, tag="lv2")
                nc.scalar.activation(lv2[:], var2[:], AF.Ln, bias=eps_t[:])
                rsig2 = small2p.tile([1, R], F16, tag="rsig2")
                nc.scalar.activation(rsig2[:], lv2[:], AF.Exp, scale=-0.5)

                bmu2_ps = ps_bc2.tile([128, R], F32, tag="bmu2")
                nc.tensor.matmul(
                    bmu2_ps[:], ones_row[:], mu2[:], start=True, stop=True
                )
                brs2_ps = ps_bc2.tile([128, R], F32, tag="brs2")
                nc.tensor.matmul(
                    brs2_ps[:], ones_row[:], rsig2[:], start=True, stop=True
                )

                g2p = constp.tile([128, H // 2], F32, tag="g2p")
                nc.sync.dma_start(g2p[:], g2_ext.ap().rearrange("(p q) -> q p", q=128))
                b2p = constp.tile([128, H // 2], F32, tag="b2p")
                nc.sync.dma_start(b2p[:], b2_ext.ap().rearrange("(p q) -> q p", q=128))

                ln2o = []
                for p in range(H // 2):
                    t1 = tmp2p.tile([128, R], F32, tag="l2t1")
                    nc.vector.tensor_tensor(
                        t1[:], opair[p][:], bmu2_ps[:], op=ALU.subtract
                    )
                    t2 = tmp2p.tile([128, R], F32, tag="l2t2")
                    nc.vector.tensor_tensor(t2[:], t1[:], brs2_ps[:], op=ALU.mult)
                    t3 = ln2op.tile([128, R], F16, tag=f"ln2o{p}", name=f"ln2o{p}")
                    nc.vector.tensor_scalar(
                        t3[:], t2[:], g2p[:, p : p + 1], b2p[:, p : p + 1],
                        op0=ALU.mult, op1=ALU.add,
                    )
                    ln2o.append(t3)

                # h-outer projection: matmuls for pair p start as soon as
                # ln2o[p] exists, overlapping the LN2 apply chain
                for half in range(2):
                    yps_t = [
                        ps_y.tile([128, R], F32, tag="yps", name=f"yps{half}_{i}")
                        for i in range(4)
                    ]
                    for p in range(H // 2):
                        for i in range(4):
                            of = 4 * half + i
                            nc.tensor.matmul(
                                yps_t[i][:],
                                wp_sb[p][:, 128 * of : 128 * (of + 1)],
                                ln2o[p][:],
                                start=(p == 0),
                                stop=(p == H // 2 - 1),
                            )
                    for i in range(4):
                        of = 4 * half + i
                        ysb = yevp.tile([128, R], F32, tag="yev")
                        nc.scalar.activation(
                            ysb[:], yps_t[i][:], AF.Identity, bias=bp[:, of : of + 1]
                        )
                        nc.sync.dma_start(
                            out_ext[128 * of : 128 * (of + 1), :], ysb[:]
                        )

    nc.compile()
    return nc


def kernel(x, ln1_g, ln1_b, w_qk, w_v, ln2_g, ln2_b, w_proj, b_proj):
    global _cached_nc, last_exec_time_ns
    if _cached_nc is None:
        _cached_nc = _build()
    nc = _cached_nc

    xr = np.asarray(x, np.float32).reshape(B * N, C).astype(np.float16)
    shared = {
        "w_qk": np.asarray(w_qk, np.float32).astype(np.float16),
        "w_v": np.asarray(w_v, np.float32).astype(np.float16),
        "w_proj": np.asarray(w_proj, np.float32).astype(np.float16),
        "ln1_g": np.ascontiguousarray(ln1_g, np.float32),
        "ln1_b": np.ascontiguousarray(ln1_b, np.float32),
        "ln2_g": np.ascontiguousarray(ln2_g, np.float32),
        "ln2_b": np.ascontiguousarray(ln2_b, np.float32),
        "b_proj": np.ascontiguousarray(b_proj, np.float32),
    }
    in_maps = []
    for i in range(NCORES):
        xT_i = np.ascontiguousarray(xr[R * i : R * (i + 1), :].T)
        in_maps.append({"xT": xT_i, **shared})

    res = run_bass_kernel_spmd(nc, in_maps, core_ids=list(range(NCORES)))
    last_exec_time_ns = res.exec_time_ns

    y = np.empty((B * N, C), np.float32)
    for i in range(NCORES):
        y[R * i : R * (i + 1), :] = res.results[i]["out"].T
    return y.reshape(B, N, C)


# revision 17
# speedup vs baseline: 2.1337x; 1.0875x over previous
"""Distributed Trainium2 kernel for the dense-transformer attention block:

    xn = LN(x); q,k = xn @ w_qk; v = xn @ w_v
    out = softmax(q k^T / sqrt(hd)) v ; out = LN(out) @ w_proj + b_proj

Sharding: the (B=2, N=2048) token axis is flattened to 4096 rows, 512 per
core (cores 0-3 own batch 0, cores 4-7 batch 1).  Each core runs LN1 and
the K/V projections on its rows, all-gathers K^T and V (fp16) inside its
4-core batch group, computes all 16 heads of attention for its own 512
query rows, then LN2 and the output projection — every FLOP except the
gathers is done exactly once across the chip.

On-chip data lives in a transposed ("feature on partition") layout: the
host feeds x^T per core and transposes the returned y^T back (free).
Matmul operands are fp16 (full PE rate + fast weight load); PSUM
accumulation and LayerNorm statistics stay fp32.  Softmax denominators
come free from a ones-column appended to V (row 64 of the AV output);
no max-subtraction is needed because scores are ~N(0,1).
"""

import numpy as np

import concourse.bass as bass
import concourse.mybir as mybir
import concourse.tile as tile
from concourse import bacc
from concourse.bass_utils import run_bass_kernel_spmd

B, N, C = 2, 2048, 1024
H, HD = 16, 64
NCORES = 8
R = (B * N) // NCORES  # 512 rows per core
GROUP = NCORES // B  # 4 cores per batch
NKC = (GROUP * R) // 128  # 16 k-chunks of 128
EPS = 1e-5
SCALE = HD**-0.5

F32 = mybir.dt.float32
F16 = mybir.dt.float16
AF = mybir.ActivationFunctionType
ALU = mybir.AluOpType

CT = C // 128  # 8 C tiles

last_exec_time_ns = None
_cached_nc = None


def _build():
    nc = bacc.Bacc("TRN2", target_bir_lowering=False, debug=False, num_devices=NCORES)

    xT_ext = nc.dram_tensor("xT", [C, R], F16, kind="ExternalInput")
    wqk_ext = nc.dram_tensor("w_qk", [C, 2 * C], F16, kind="ExternalInput")
    wv_ext = nc.dram_tensor("w_v", [C, C], F16, kind="ExternalInput")
    wp_ext = nc.dram_tensor("w_proj", [C, C], F16, kind="ExternalInput")
    g1_ext = nc.dram_tensor("ln1_g", [C], F32, kind="ExternalInput")
    b1_ext = nc.dram_tensor("ln1_b", [C], F32, kind="ExternalInput")
    g2_ext = nc.dram_tensor("ln2_g", [C], F32, kind="ExternalInput")
    b2_ext = nc.dram_tensor("ln2_b", [C], F32, kind="ExternalInput")
    bp_ext = nc.dram_tensor("b_proj", [C], F32, kind="ExternalInput")
    out_ext = nc.dram_tensor("out", [C, R], F32, kind="ExternalOutput")

    # k^T split in two halves so the first gather starts earlier
    k_in = [
        nc.dram_tensor(f"k_in{i}", [C // 2, R], F16) for i in range(2)
    ]  # k^T, feature-major
    k_out = [nc.dram_tensor(f"k_out{i}", [GROUP, C // 2, R], F16) for i in range(2)]
    # v + ones col per head, split in two halves (heads 0-7 / 8-15)
    v_in = [nc.dram_tensor(f"v_in{i}", [R, H // 2, HD + 1], F16) for i in range(2)]
    v_out = [
        nc.dram_tensor(f"v_out{i}", [GROUP, R, H // 2, HD + 1], F16) for i in range(2)
    ]

    with tile.TileContext(nc) as tc:
        with (
            tc.tile_pool(name="const", bufs=1) as constp,
            tc.tile_pool(name="qT", bufs=1) as qTp,
            tc.tile_pool(name="o", bufs=1) as op_,
            tc.tile_pool(name="acc", bufs=1) as accp,
        ):
            s2acc = accp.tile([1, R], F32, tag="s2acc")
            nc.vector.memset(s2acc[:], 0.0)
            q2acc = accp.tile([1, R], F32, tag="q2acc")
            nc.vector.memset(q2acc[:], 0.0)
            ones_col = constp.tile([128, 1], F16, tag="onesc")
            nc.vector.memset(ones_col[:], 1.0)
            ones_row = constp.tile([1, 128], F16, tag="onesr")
            nc.vector.memset(ones_row[:], 1.0)
            eps_t = constp.tile([1, 1], F32, tag="epsc")
            nc.sync.dma_start(
                eps_t[:], nc.inline_tensor(np.full((1, 1), EPS, np.float32), "epsc").ap()
            )
            g1 = constp.tile([128, CT], F32, tag="g1")
            nc.sync.dma_start(g1[:], g1_ext.ap().rearrange("(c p) -> p c", p=128))
            b1 = constp.tile([128, CT], F32, tag="b1")
            nc.sync.dma_start(b1[:], b1_ext.ap().rearrange("(c p) -> p c", p=128))
            g2h = constp.tile([64, H], F32, tag="g2h")
            nc.sync.dma_start(g2h[:], g2_ext.ap().rearrange("(h p) -> p h", p=64))
            b2h = constp.tile([64, H], F32, tag="b2h")
            nc.sync.dma_start(b2h[:], b2_ext.ap().rearrange("(h p) -> p h", p=64))
            bp = constp.tile([128, CT], F32, tag="bp")
            nc.sync.dma_start(bp[:], bp_ext.ap().rearrange("(c p) -> p c", p=128))

            qT = [qTp.tile([128, R], F16, tag=f"qT{p}", name=f"qT{p}") for p in range(H // 2)]
            opair = [
                op_.tile([128, R], F16, tag=f"op{p}", name=f"op{p}")
                for p in range(H // 2)
            ]

            # ---------------- Phase 1: LN1 + K/V/Q projections ----------
            with (
                tc.tile_pool(name="xn", bufs=1) as xnp,
                tc.tile_pool(name="w1", bufs=1) as w1p,
                tc.tile_pool(name="tmp1", bufs=3) as tmp1p,
                tc.tile_pool(name="small1", bufs=1) as small1p,
                tc.tile_pool(name="ev1", bufs=4) as ev1p,
                tc.tile_pool(name="xt", bufs=1) as xtp,
                tc.tile_pool(name="ps_stat", bufs=1, space="PSUM") as ps_stat,
                tc.tile_pool(name="ps_bc", bufs=1, space="PSUM") as ps_bc,
                tc.tile_pool(name="ps_mm", bufs=3, space="PSUM") as ps_mm,
            ):
                xt = []
                for c in range(CT):
                    t = xtp.tile([128, R], F16, tag=f"xt{c}")
                    nc.sync.dma_start(t[:], xT_ext[128 * c : 128 * (c + 1), :])
                    xt.append(t)

                # resident fp16 weights; K-projection slabs first so the
                # k^T matmuls (feeding the first all-gather) never wait
                wk_sb = []
                wq_sb = []
                wv_sb = []
                for c in range(CT):
                    t = w1p.tile([128, C], F16, tag=f"wk{c}", name=f"wk{c}")
                    nc.sync.dma_start(t[:], wqk_ext[128 * c : 128 * (c + 1), C:])
                    wk_sb.append(t)
                for c in range(CT):
                    t = w1p.tile([128, C], F16, tag=f"wv{c}", name=f"wv{c}")
                    nc.sync.dma_start(t[:], wv_ext[128 * c : 128 * (c + 1), :])
                    wv_sb.append(t)
                for c in range(CT):
                    t = w1p.tile([128, C], F16, tag=f"wq{c}", name=f"wq{c}")
                    nc.sync.dma_start(t[:], wqk_ext[128 * c : 128 * (c + 1), 0:C])
                    wq_sb.append(t)

                # LN1 stats: sums of x and x^2 over C (partition axis)
                sx_ps = ps_stat.tile([1, R], F32, tag="sx")
                sq_ps = ps_stat.tile([1, R], F32, tag="sq")
                for c in range(CT):
                    nc.tensor.matmul(
                        sx_ps[:], ones_col[:], xt[c][:], start=(c == 0), stop=(c == CT - 1)
                    )
                for c in range(CT):
                    xsq = tmp1p.tile([128, R], F16, tag="xsq")
                    if c % 2 == 0:
                        nc.scalar.activation(xsq[:], xt[c][:], AF.Square)
                    else:
                        nc.vector.tensor_tensor(xsq[:], xt[c][:], xt[c][:], op=ALU.mult)
                    nc.tensor.matmul(
                        sq_ps[:], ones_col[:], xsq[:], start=(c == 0), stop=(c == CT - 1)
                    )

                mu = small1p.tile([1, R], F16, tag="mu")
                nc.vector.tensor_scalar_mul(mu[:], sx_ps[:], 1.0 / C)
                m2 = small1p.tile([1, R], F32, tag="m2")
                nc.vector.tensor_scalar_mul(m2[:], sq_ps[:], 1.0 / C)
                musq = small1p.tile([1, R], F32, tag="musq")
                nc.vector.tensor_tensor(musq[:], mu[:], mu[:], op=ALU.mult)
                var = small1p.tile([1, R], F32, tag="var")
                nc.vector.tensor_tensor(var[:], m2[:], musq[:], op=ALU.subtract)
                lv = small1p.tile([1, R], F32, tag="lv")
                nc.scalar.activation(lv[:], var[:], AF.Ln, bias=eps_t[:])
                rsig = small1p.tile([1, R], F16, tag="rsig")
                nc.scalar.activation(rsig[:], lv[:], AF.Exp, scale=-0.5)

                bmu_ps = ps_bc.tile([128, R], F32, tag="bmu")
                nc.tensor.matmul(bmu_ps[:], ones_row[:], mu[:], start=True, stop=True)
                brs_ps = ps_bc.tile([128, R], F32, tag="brs")
                nc.tensor.matmul(brs_ps[:], ones_row[:], rsig[:], start=True, stop=True)

                xn = []
                for c in range(CT):
                    t1 = tmp1p.tile([128, R], F32, tag="lt1")
                    nc.vector.tensor_tensor(t1[:], xt[c][:], bmu_ps[:], op=ALU.subtract)
                    t2 = tmp1p.tile([128, R], F32, tag="lt2")
                    nc.vector.tensor_tensor(t2[:], t1[:], brs_ps[:], op=ALU.mult)
                    t3 = xnp.tile([128, R], F16, tag=f"xn{c}")
                    nc.vector.tensor_scalar(
                        t3[:], t2[:], g1[:, c : c + 1], b1[:, c : c + 1],
                        op0=ALU.mult, op1=ALU.add,
                    )
                    xn.append(t3)

                # k^T and v computed and gathered in interleaved halves:
                # AG(k0) AG(v0) AG(k1) AG(v1), so score work for pairs 0-3
                # overlaps AG(v0), AV work for pairs 0-3 overlaps AG(k1), ...
                for half in range(2):
                    for kfh in range(CT // 2):
                        kf = half * (CT // 2) + kfh
                        kps = ps_mm.tile([128, R], F32, tag="kvps")
                        for c in range(CT):
                            nc.tensor.matmul(
                                kps[:],
                                wk_sb[c][:, 128 * kf : 128 * (kf + 1)],
                                xn[c][:],
                                start=(c == 0),
                                stop=(c == CT - 1),
                            )
                        ksb = ev1p.tile([128, R], F16, tag="kev")
                        nc.scalar.copy(ksb[:], kps[:])
                        nc.sync.dma_start(
                            k_in[half][128 * kfh : 128 * (kfh + 1), :], ksb[:]
                        )
                    nc.gpsimd.collective_compute(
                        "AllGather",
                        ALU.bypass,
                        ins=[k_in[half][:]],
                        outs=[k_out[half][:]],
                        replica_groups=[[0, 1, 2, 3], [4, 5, 6, 7]],
                    )
                    # v half: heads 8*half .. 8*half+8
                    for rt in range(R // 128):
                        vps = ps_mm.tile([128, R], F32, tag="kvps")
                        for c in range(CT):
                            nc.tensor.matmul(
                                vps[:],
                                xn[c][:, 128 * rt : 128 * (rt + 1)],
                                wv_sb[c][:, 512 * half : 512 * (half + 1)],
                                start=(c == 0),
                                stop=(c == CT - 1),
                            )
                        vsb = ev1p.tile([128, 8, HD + 1], F16, tag="vev")
                        nc.scalar.copy(
                            vsb[:, :, 0:HD],
                            vps[:].rearrange("p (h d) -> p h d", d=HD),
                        )
                        nc.vector.memset(vsb[:, :, HD : HD + 1], 1.0)
                        nc.sync.dma_start(
                            v_in[half][128 * rt : 128 * (rt + 1), :, :], vsb[:]
                        )
                    nc.gpsimd.collective_compute(
                        "AllGather",
                        ALU.bypass,
                        ins=[v_in[half][:]],
                        outs=[v_out[half][:]],
                        replica_groups=[[0, 1, 2, 3], [4, 5, 6, 7]],
                    )

                # q^T = w_q^T @ xn^T -> [C, R] (overlaps the all-gathers)
                for qf in range(CT):
                    qps = ps_mm.tile([128, R], F32, tag="kvps")
                    for c in range(CT):
                        nc.tensor.matmul(
                            qps[:],
                            wq_sb[c][:, 128 * qf : 128 * (qf + 1)],
                            xn[c][:],
                            start=(c == 0),
                            stop=(c == CT - 1),
                        )
                    nc.scalar.copy(qT[qf][:], qps[:])

            # ---------------- Phase 2: attention -------------------------
            # Wave w covers head-pairs 4w..4w+3: scores depend on AG(k_w),
            # AV on AG(v_w); emission order (sc wave0, av wave0, sc wave1,
            # av wave1) keeps the PE busy through every gather window.
            with (
                tc.tile_pool(name="kpair", bufs=4) as kpp,
                tc.tile_pool(name="vpair", bufs=4) as vpp,
                tc.tile_pool(name="att", bufs=32) as attp,
                tc.tile_pool(name="rec", bufs=6) as recp,
                tc.tile_pool(name="oh", bufs=4) as ohp,
                tc.tile_pool(name="ps_sc", bufs=2, space="PSUM") as ps_sc,
                tc.tile_pool(name="ps_av", bufs=3, space="PSUM") as ps_av,
                tc.tile_pool(name="ps_srot", bufs=1, space="PSUM") as ps_srot,
            ):
                NP = H // 2
                att = [None] * NP

                kpair_t = [None] * NP
                vpair_t = [None] * NP

                def load_kpair(p):
                    kpair_t[p] = kpp.tile(
                        [128, GROUP * R], F16, tag="kpair", name=f"kp{p}"
                    )
                    half, ph = divmod(p, CT // 2)
                    nc.sync.dma_start(
                        kpair_t[p][:].rearrange("p (r n) -> p r n", r=GROUP),
                        k_out[half][:, 128 * ph : 128 * (ph + 1), :].rearrange(
                            "r p n -> p r n"
                        ),
                    )

                def load_vpair(p):
                    half, ph = divmod(p, CT // 2)
                    vpair_t[p] = vpp.tile(
                        [128, NKC, 2 * (HD + 1)], F16, tag="vpair", name=f"vp{p}"
                    )
                    nc.sync.dma_start(
                        vpair_t[p][:],
                        v_out[half][:]
                        .rearrange("r (j p) h d -> p (r j) (h d)", p=128)[
                            :, :, (HD + 1) * 2 * ph : (HD + 1) * 2 * (ph + 1)
                        ],
                    )

                def scores_pair(p):
                    kpair = kpair_t[p]
                    att[p] = [[], []]
                    for g in range(NKC // 2):
                        scp = [None, None]
                        for hi in range(2):
                            scp[hi] = ps_sc.tile(
                                [128, 1024], F32, tag="scps", name=f"sc{p}_{g}_{hi}"
                            )
                        for cc in range(2):
                            j = 2 * g + cc
                            for hi in range(2):
                                base = 64 * hi
                                nc.tensor.matmul(
                                    scp[hi][:, 512 * cc : 512 * (cc + 1)],
                                    kpair[base : base + 64, 128 * j : 128 * (j + 1)],
                                    qT[p][base : base + 64, :],
                                    start=True,
                                    stop=True,
                                )
                        for hi in range(2):
                            at = attp.tile([128, 1024], F16, tag="att")
                            nc.scalar.activation(at[:], scp[hi][:], AF.Exp, scale=SCALE)
                            att[p][hi].append(at)

                def av_pair(p):
                    vpair = vpair_t[p]
                    for hi in range(2):
                        h = 2 * p + hi
                        avps = ps_av.tile([HD + 1, R], F32, tag="avps", name=f"av{h}")
                        for j in range(NKC):
                            nc.tensor.matmul(
                                avps[:],
                                vpair[:, j, (HD + 1) * hi : (HD + 1) * (hi + 1)],
                                att[p][hi][j // 2][:, 512 * (j % 2) : 512 * (j % 2 + 1)],
                                start=(j == 0),
                                stop=(j == NKC - 1),
                            )
                        # two quick copies release the AV bank, the rest of
                        # the softmax-normalization chain runs off-psum
                        oh = ohp.tile([64, R], F16, tag="oh")
                        nc.vector.tensor_copy(oh[:], avps[0:HD, :])
                        den = recp.tile([1, R], F32, tag="den")
                        nc.vector.tensor_copy(den[:], avps[HD : HD + 1, :])
                        recf = recp.tile([1, R], F32, tag="recf")
                        with nc.allow_low_precision(reason="softmax denom bcast"):
                            nc.vector.reciprocal_approx_fast(recf[:], den[:])
                        rec = recp.tile([1, R], F16, tag="rec")
                        nc.vector.tensor_copy(rec[:], recf[:])
                        # broadcast 1/denom across partitions on the idle
                        # GpSimd engine (no PSUM, no PE involvement)
                        S = recp.tile([64, R], F16, tag="Sb")
                        nc.gpsimd.partition_broadcast(S[:], rec[:])
                        nc.vector.tensor_tensor(oh[:], oh[:], S[:], op=ALU.mult)
                        nc.sync.dma_start(opair[p][64 * hi : 64 * (hi + 1), :], oh[:])
                    # LN2 statistics accumulate (SBUF) while attention runs
                    s2p = ps_srot.tile([1, R], F32, tag="s2p", name=f"s2p{p}")
                    nc.tensor.matmul(
                        s2p[:], ones_col[:], opair[p][:], start=True, stop=True
                    )
                    nc.vector.tensor_tensor(s2acc[:], s2acc[:], s2p[:], op=ALU.add)
                    osq = recp.tile([128, R], F16, tag="osq")
                    nc.vector.tensor_tensor(osq[:], opair[p][:], opair[p][:], op=ALU.mult)
                    q2p = ps_srot.tile([1, R], F32, tag="s2p", name=f"q2p{p}")
                    nc.tensor.matmul(q2p[:], ones_col[:], osq[:], start=True, stop=True)
                    nc.vector.tensor_tensor(q2acc[:], q2acc[:], q2p[:], op=ALU.add)
                    att[p] = None

                for wave in range(2):
                    for p in range(4 * wave, 4 * wave + 4):
                        load_kpair(p)
                    for p in range(4 * wave, 4 * wave + 4):
                        scores_pair(p)
                    for p in range(4 * wave, 4 * wave + 4):
                        load_vpair(p)
                    for p in range(4 * wave, 4 * wave + 4):
                        av_pair(p)

            # ---------------- Phase 3: LN2 + projection ------------------
            with (
                tc.tile_pool(name="wp", bufs=1) as wpp,
                tc.tile_pool(name="tmp2", bufs=4) as tmp2p,
                tc.tile_pool(name="small2", bufs=1) as small2p,
                tc.tile_pool(name="ln2o", bufs=1) as ln2op,
                tc.tile_pool(name="yev", bufs=3) as yevp,
                tc.tile_pool(name="ps_bc2", bufs=1, space="PSUM") as ps_bc2,
                tc.tile_pool(name="ps_y", bufs=4, space="PSUM") as ps_y,
            ):
                wp_sb = []
                for p in range(H // 2):
                    t = wpp.tile([128, C], F16, tag=f"wp{p}", name=f"wpp{p}")
                    nc.sync.dma_start(t[:], wp_ext[128 * p : 128 * (p + 1), :])
                    wp_sb.append(t)

                mu2 = small2p.tile([1, R], F16, tag="mu2")
                nc.vector.tensor_scalar_mul(mu2[:], s2acc[:], 1.0 / C)
                m22 = small2p.tile([1, R], F32, tag="m22")
                nc.vector.tensor_scalar_mul(m22[:], q2acc[:], 1.0 / C)
                musq2 = small2p.tile([1, R], F32, tag="musq2")
                nc.vector.tensor_tensor(musq2[:], mu2[:], mu2[:], op=ALU.mult)
                var2 = small2p.tile([1, R], F32, tag="var2")
                nc.vector.tensor_tensor(var2[:], m22[:], musq2[:], op=ALU.subtract)
                lv2 = small2p.tile([1, R], F32, tag="lv2")
                nc.scalar.activation(lv2[:], var2[:], AF.Ln, bias=eps_t[:])
                rsig2 = small2p.tile([1, R], F16, tag="rsig2")
                nc.scalar.activation(rsig2[:], lv2[:], AF.Exp, scale=-0.5)

                bmu2_ps = ps_bc2.tile([128, R], F32, tag="bmu2")
                nc.tensor.matmul(
                    bmu2_ps[:], ones_row[:], mu2[:], start=True, stop=True
                )
                brs2_ps = ps_bc2.tile([128, R], F32, tag="brs2")
                nc.tensor.matmul(
                    brs2_ps[:], ones_row[:], rsig2[:], start=True, stop=True
                )

                g2p = constp.tile([128, H // 2], F32, tag="g2p")
                nc.sync.dma_start(g2p[:], g2_ext.ap().rearrange("(p q) -> q p", q=128))
                b2p = constp.tile([128, H // 2], F32, tag="b2p")
                nc.sync.dma_start(b2p[:], b2_ext.ap().rearrange("(p q) -> q p", q=128))

                ln2o = []
                for p in range(H // 2):
                    t1 = tmp2p.tile([128, R], F32, tag="l2t1")
                    nc.vector.tensor_tensor(
                        t1[:], opair[p][:], bmu2_ps[:], op=ALU.subtract
                    )
                    t2 = tmp2p.tile([128, R], F32, tag="l2t2")
                    nc.vector.tensor_tensor(t2[:], t1[:], brs2_ps[:], op=ALU.mult)
                    t3 = ln2op.tile([128, R], F16, tag=f"ln2o{p}", name=f"ln2o{p}")
                    nc.vector.tensor_scalar(
                        t3[:], t2[:], g2p[:, p : p + 1], b2p[:, p : p + 1],
                        op0=ALU.mult, op1=ALU.add,
                    )
                    ln2o.append(t3)

                # h-outer projection: matmuls for pair p start as soon as
                # ln2o[p] exists, overlapping the LN2 apply chain
                for half in range(2):
                    yps_t = [
                        ps_y.tile([128, R], F32, tag="yps", name=f"yps{half}_{i}")
                        for i in range(4)
                    ]
                    for p in range(H // 2):
                        for i in range(4):
                            of = 4 * half + i
                            nc.tensor.matmul(
                                yps_t[i][:],
                                wp_sb[p][:, 128 * of : 128 * (of + 1)],
                                ln2o[p][:],
                                start=(p == 0),
                                stop=(p == H // 2 - 1),
                            )
                    for i in range(4):
                        of = 4 * half + i
                        ysb = yevp.tile([128, R], F32, tag="yev")
                        nc.scalar.activation(
                            ysb[:], yps_t[i][:], AF.Identity, bias=bp[:, of : of + 1]
                        )
                        nc.sync.dma_start(
                            out_ext[128 * of : 128 * (of + 1), :], ysb[:]
                        )

    nc.compile()
    return nc


def kernel(x, ln1_g, ln1_b, w_qk, w_v, ln2_g, ln2_b, w_proj, b_proj):
    global _cached_nc, last_exec_time_ns
    if _cached_nc is None:
        _cached_nc = _build()
    nc = _cached_nc

    xr = np.asarray(x, np.float32).reshape(B * N, C).astype(np.float16)
    shared = {
        "w_qk": np.asarray(w_qk, np.float32).astype(np.float16),
        "w_v": np.asarray(w_v, np.float32).astype(np.float16),
        "w_proj": np.asarray(w_proj, np.float32).astype(np.float16),
        "ln1_g": np.ascontiguousarray(ln1_g, np.float32),
        "ln1_b": np.ascontiguousarray(ln1_b, np.float32),
        "ln2_g": np.ascontiguousarray(ln2_g, np.float32),
        "ln2_b": np.ascontiguousarray(ln2_b, np.float32),
        "b_proj": np.ascontiguousarray(b_proj, np.float32),
    }
    in_maps = []
    for i in range(NCORES):
        xT_i = np.ascontiguousarray(xr[R * i : R * (i + 1), :].T)
        in_maps.append({"xT": xT_i, **shared})

    res = run_bass_kernel_spmd(nc, in_maps, core_ids=list(range(NCORES)))
    last_exec_time_ns = res.exec_time_ns

    y = np.empty((B * N, C), np.float32)
    for i in range(NCORES):
        y[R * i : R * (i + 1), :] = res.results[i]["out"].T
    return y.reshape(B, N, C)
